# revision 1
# baseline (speedup 1.0000x reference)
"""Trainium2 Bass kernel for nn_Net_63754494542044.

Data-parallel over 8 NeuronCores (8 B-samples each). Host pre-packs
conv1 im2col / conv weights / RoIAlign grid tables; device runs
conv1 -> conv2 -> RoIAlign gather+bilinear -> fc0/emb/red -> 8 GNN rollouts.
"""
import sys
sys.path.insert(0, '/opt/trn_rl_repo')
import numpy as np
from contextlib import ExitStack
import concourse.bass as bass
import concourse.tile as tile
from concourse import mybir
from concourse.bass_utils import run_bass_kernel_spmd

# Walrus wait-slot limits: CTRL-encoded (Drain/NoOp) = 1; others appear
# limited too on this build -- split conservatively.
def split_drain_waits(nc, max_waits=1, max_waits_other=1):
    for fn in nc.m.functions:
        for bb in fn.blocks:
            insts = bb.instructions
            i = 0
            while i < len(insts):
                inst = insts[i]
                si = getattr(inst, 'sync_info', None)
                lim = max_waits if isinstance(inst, (mybir.InstDrain, mybir.InstNoOp)) else max_waits_other
                if si is not None and si.on_wait and len(si.on_wait) > lim:
                    waits = list(si.on_wait)
                    keep = waits[-lim:]
                    extra = waits[:-lim]
                    new_nops = []
                    for k in range(0, len(extra), max_waits):
                        chunk = extra[k:k + max_waits]
                        nop = mybir.InstNoOp(
                            name=nc.get_next_instruction_name(),
                            engine=inst.engine,
                        )
                        nop.sync_info = mybir.SyncInfo(on_wait=chunk, on_update=[])
                        nc.register_instruction(nop)
                        new_nops.append(nop)
                    inst.sync_info = mybir.SyncInfo(on_wait=keep, on_update=list(si.on_update))
                    insts[i:i] = new_nops
                    i += len(new_nops)
                i += 1


import os
FP8_CONV = os.environ.get('BASSK_FP8', '1') == '1'

B, T, N = 64, 4, 6
IMG, CIN = 128, 3
VE, D, P = 64, 256, 4
SCALE = 0.25
NCORE = 8
BC = B // NCORE          # 8 samples per core
NIMG = BC * T            # 32 images per core
NROI = BC * T * N        # 192 rois per core
NROW = BC * N            # 48 gnn rows per core
NPT = NROI * 16          # 3072 sample points per core
NG = 24                  # gather groups


# ---------------- conv1 im2col (host) ----------------
# conv1: 3->64, 3x3, stride2, SAME on 128x128 -> 64x64.
# 2-px-packed output: out pair (oy, j) covers ox = 2j, 2j+1.
# K=45 rows: (rowtap rt in 0..2) x (coltap ct in 0..4) x (ci in 0..2)
#   input row for out oy: rt0: 2*oy-1, rt1: 2*oy, rt2: 2*oy+1
#   input col for out pair j: ct: 4j-1, 4j, 4j+1, 4j+2, 4j+3
def conv1_im2col_host(x):  # x [nimg, 3, 128, 128] fp32
    nimg = x.shape[0]
    xp = np.pad(x, ((0, 0), (0, 0), (0, 1), (0, 1)))  # SAME stride2: pad bottom/right only
    cols = np.empty((45, nimg, 64, 32), np.float32)
    k = 0
    for rt in range(3):
        for ct in range(5):
            for ci in range(3):
                # row = 2*oy + rt ; col = 4*j + ct
                cols[k] = xp[:, ci, rt:rt + 127:2, ct:ct + 125:4]
                k += 1
    return cols  # [45, nimg, 64, 32]


def conv1_weights_host(w_conv1):  # [64, 3, 3, 3]
    # W2 [45, 128]: col m = px*64 + oc ... out(oy, 2j+px) uses taps:
    #   orig tap (dy, dx): input row 2oy+dy-1 -> rt = dy ; input col 2(2j+px)+dx-1 = 4j + (2px+dx-1) -> ct = 2px+dx-1
    W2 = np.zeros((45, 128), np.float32)
    for px in range(2):
        for oc in range(64):
            m = px * 64 + oc
            for dy in range(3):
                for dx in range(3):
                    ct = 2 * px + dx
                    assert 0 <= ct <= 4  # input col = 4j + ct (no left pad)
                    for ci in range(3):
                        W2[(dy * 5 + ct) * 3 + ci, m] = w_conv1[oc, ci, dy, dx]
    return W2


def conv1_host(x, w_conv1, b_conv1):
    """Mirror of device conv1: returns feat1 [nimg, 64, 64, 64] (pre-relu + bias)."""
    cols = conv1_im2col_host(x)          # [45, nimg, 64, 32]
    W2 = conv1_weights_host(w_conv1)     # [45, 128]
    out = np.einsum('kf,kc->cf', cols.reshape(45, -1), W2)  # [128, nimg*64*32]
    out = out.reshape(2, 64, -1, 64, 32)  # [px, oc, img, oy, j]
    feat1 = np.empty((x.shape[0], 64, 64, 64), np.float32)
    feat1[..., 0::2] = np.transpose(out[0], (1, 0, 2, 3))
    feat1[..., 1::2] = np.transpose(out[1], (1, 0, 2, 3))
    feat1 += b_conv1[None, :, None, None]
    return feat1


# ---------------- conv2 weights (host) ----------------
# feat1_ph partitions: (px_in*64 + ci'), free (img, py, Y, X) halo X,Y in -1..31.
# conv2 out pair (oy2, j2): outs o1=2*j2, o2=2*j2+1 ; M col = pxo*64 + oc.
# 9 matmuls: rowtap r in {py0[Y], py1[Y-1], py1[Y]} x colgrp g in {X=j2 pair(K128), X=j2+?...}
# col groups: g0: pair (px0[Xa], px1[Xa]) Xa = j2? ... define by original dx:
#   out ox2: input x = 2*ox2 + dx - 1
#   for o1=2j2: x = 4j2-1, 4j2, 4j2+1 -> (px,X): (1, 2j2-1), (0, 2j2), (1, 2j2)
#   for o2=2j2+1: x = 4j2+1, 4j2+2, 4j2+3 -> (1, 2j2), (0, 2j2+1), (1, 2j2+1)
# X taps: px0: {2j2, 2j2+1} ; px1: {2j2-1, 2j2, 2j2+1}
# col groups (relative X offset from base 2j2):
#   gA: K128 = (px0[2j2], px1[2j2])        -> X offset 0, both phases
#   gB: K128 = (px0[2j2+1], px1[2j2+1])    -> X offset +1, both phases
#   gC: K64  = px1[2j2-1]                  -> X offset -1, px1 only
# rowtaps r (input y = 2*oy2 + dy - 1):
#   dy0: y = 2oy2-1 -> (py1, Y=oy2-1) ; dy1: y=2oy2 -> (py0, Y=oy2) ; dy2: y=2oy2+1 -> (py1, Y=oy2)
def conv2_weights_host(w_conv2):  # [64, 64, 3, 3]
    # Wb[r][g]: gA/gB: [128, 128] (partition = pxi*64+ci), gC: [64, 128]
    # dy maps to rowtap r directly (r=0: dy=0 ; r=1: dy=1 ; r=2: dy=2)
    Wb = [[np.zeros((128, 128), np.float32) for _ in range(2)] + [np.zeros((64, 128), np.float32)]
          for _ in range(3)]
    for pxo in range(2):          # which output in the pair (o = 2j2+pxo)
        for oc in range(64):
            m = pxo * 64 + oc
            for dy in range(3):
                for dx in range(3):
                    x_off = 2 * pxo + dx     # input x = 4j2 + x_off, x_off in 0..4
                    pxi = x_off % 2
                    Xrel = x_off // 2        # in {0, 1, 2}
                    for ci in range(64):
                        if Xrel < 2:
                            Wb[dy][Xrel][pxi * 64 + ci, m] += w_conv2[oc, ci, dy, dx]
                        else:
                            assert pxi == 0
                            Wb[dy][2][ci, m] += w_conv2[oc, ci, dy, dx]
    return Wb


def conv2_host(feat1r, w_conv2, b_conv2):
    """feat1r: relu'd feat1 [nimg, 64, 64, 64]. Returns feat2 [nimg, 64, 32, 32] pre-relu."""
    nimg = feat1r.shape[0]
    # build feat1_ph with halo: [128 part (pxi*64+ci), img, py, Y(-1..31), X(-1..31)]
    ph = np.zeros((128, nimg, 2, 33, 33), np.float32)  # halo at Y=32, X=32
    f = feat1r  # [img, ci, y, x]
    for pxi in range(2):
        for py in range(2):
            ph[pxi * 64:pxi * 64 + 64, :, py, :32, :32] = np.transpose(
                f[:, :, py::2, pxi::2], (1, 0, 2, 3))
    Wb = conv2_weights_host(w_conv2)
    out = np.zeros((128, nimg, 32, 16), np.float32)  # [(pxo,oc), img, oy2, j2]
    # rowtap dy: input y = 2*oy2 + dy -> (py = dy&1, Y = oy2 + dy//2)
    for dy in range(3):
        py, Yoff = dy % 2, dy // 2
        for g in range(3):
            W = Wb[dy][g]
            Ysl = slice(Yoff, Yoff + 32)
            Xidx = g + 2 * np.arange(16)   # X = 2*j2 + Xrel ... stored X index = that
            rhs = ph[:, :, py, Ysl, :][:, :, :, Xidx]  # [128 or .., img, 32, 16]
            if g == 2:
                rhs = rhs[:64]
            out += np.einsum('km,kijx->mijx', W, rhs)
    feat2 = np.empty((nimg, 64, 32, 32), np.float32)
    feat2[..., 0::2] = np.transpose(out[:64], (1, 0, 2, 3))
    feat2[..., 1::2] = np.transpose(out[64:], (1, 0, 2, 3))
    return feat2 + b_conv2[None, :, None, None]


# ---------------- RoIAlign grid (host) ----------------
def roi_grid_host(rois):  # rois [NROI, 5] fp32 (batch-local; bidx = local img idx)
    """Returns idx int32 [NPT, 2] (row-gather indices, row=(img,y,j2) width 128),
    weights w4 [NPT, 4] fp32 (w00,w01,w10,w11 order: (y0x0, y0x1, y1x0, y1x1)),
    parity [NPT] (x0&1)."""
    nroi = rois.shape[0]
    W = H = 32
    x1 = rois[:, 1] * SCALE; y1 = rois[:, 2] * SCALE
    x2 = rois[:, 3] * SCALE; y2 = rois[:, 4] * SCALE
    bw = np.maximum(x2 - x1, 1.0) / P
    bh = np.maximum(y2 - y1, 1.0) / P
    grid = np.arange(P, dtype=np.float32) + 0.5
    sx = x1[:, None, None] + bw[:, None, None] * grid[None, None, :]   # [R, P(py), P(px)]
    sy = y1[:, None, None] + bh[:, None, None] * grid[None, :, None]
    sx = np.broadcast_to(sx, (nroi, P, P)).reshape(-1)
    sy = np.broadcast_to(sy, (nroi, P, P)).reshape(-1)
    x0f = np.clip(np.floor(sx), 0, W - 1)
    y0f = np.clip(np.floor(sy), 0, H - 1)
    lx = np.clip(sx - x0f, 0.0, 1.0)
    ly = np.clip(sy - y0f, 0.0, 1.0)
    # clamp x0 to <= 30 adjusting lx (exact when sx>=31: both corners read col 31)
    x0 = x0f.astype(np.int32); y0 = y0f.astype(np.int32)
    hi = x0 >= 31
    x0 = np.where(hi, 30, x0); lx = np.where(hi, 1.0, lx).astype(np.float32)
    hiy = y0 >= 31
    y0 = np.where(hiy, 30, y0); ly = np.where(hiy, 1.0, ly).astype(np.float32)
    img = np.repeat(np.arange(nroi, dtype=np.int32) // N, 16)
    j2 = x0 >> 1
    par = (x0 & 1).astype(np.float32)
    idx0 = img * 512 + y0 * 16 + j2          # row idx (rows of 128 els)
    idx1 = idx0 + 16                          # y0+1 row
    w4 = np.stack([(1 - ly) * (1 - lx), (1 - ly) * lx, ly * (1 - lx), ly * lx], 1).astype(np.float32)
    return np.stack([idx0, idx1], 1).astype(np.int32), w4, par


def roi_wmat_host(rois):
    """RoIAlign as per-image matmul: sparse bilinear weights densified.
    Returns Wg [NIMG, 8, 128, 96] f32: for image i, chunk c = b*2+px
    (b = pair block 0..3, px = x parity), Wg[i, c, pair_local, n*16+pt] =
    bilinear weight of pixel (y, x) for point pt of roi n, where
    pair = y*16 + (x>>1) = b*128 + pair_local."""
    nroi = rois.shape[0]
    x1 = rois[:, 1] * SCALE; y1 = rois[:, 2] * SCALE
    x2 = rois[:, 3] * SCALE; y2 = rois[:, 4] * SCALE
    bw = np.maximum(x2 - x1, 1.0) / P
    bh = np.maximum(y2 - y1, 1.0) / P
    grid = np.arange(P, dtype=np.float32) + 0.5
    sx = x1[:, None, None] + bw[:, None, None] * grid[None, None, :]
    sy = y1[:, None, None] + bh[:, None, None] * grid[None, :, None]
    sx = np.broadcast_to(sx, (nroi, P, P)).reshape(-1)
    sy = np.broadcast_to(sy, (nroi, P, P)).reshape(-1)
    x0f = np.clip(np.floor(sx), 0, 31); y0f = np.clip(np.floor(sy), 0, 31)
    lx = np.clip(sx - x0f, 0.0, 1.0).astype(np.float32)
    ly = np.clip(sy - y0f, 0.0, 1.0).astype(np.float32)
    x0 = x0f.astype(np.int64); y0 = y0f.astype(np.int64)
    x1i = np.minimum(x0 + 1, 31); y1i = np.minimum(y0 + 1, 31)
    img = np.arange(nroi).repeat(16) // N
    col = (np.arange(nroi) % N).repeat(16) * 16 + np.tile(np.arange(16), nroi)
    Wg = np.zeros((NIMG, 8, 128, 96), np.float32)
    flat = Wg.reshape(-1)
    for w, yy, xx in (((1 - ly) * (1 - lx), y0, x0), ((1 - ly) * lx, y0, x1i),
                      (ly * (1 - lx), y1i, x0), (ly * lx, y1i, x1i)):
        pair = yy * 16 + (xx >> 1)
        c = (pair >> 7) * 2 + (xx & 1)
        idxf = ((img * 8 + c) * 128 + (pair & 127)) * 96 + col
        np.add.at(flat, idxf, w)
    return Wg


def roi_align_host(feat2r, rois):
    """Mirror of device pool-matmul -> pooled [NPT, 64] pt-major."""
    Wg = roi_wmat_host(rois)                       # [NIMG, 8, 128, 96]
    # F2c[i, c, pl, ch] = feat2r[i, ch, y, x], c = (y//8)*2 + (x&1),
    # pl = (y%8)*16 + (x>>1)
    f = feat2r.reshape(NIMG, 64, 4, 8, 16, 2)       # [i, ch, b, y8, j2, px]
    F2c = np.transpose(f, (0, 2, 5, 3, 4, 1)).reshape(NIMG, 4, 2, 128, 64)
    F2c = F2c.reshape(NIMG, 8, 128, 64)             # chunk order (b, px) ✓
    pooled = np.einsum('icpn,icpm->inm', Wg, F2c)   # [i, 96, 64]
    return pooled.reshape(NPT, 64)


# ---------------- GNN (host mirror of device algebra) ----------------
def mask_host(coor, r):
    """coor [BC, N, 2], r [BC, N] -> bigmask [NROW, NROW] fp32 block-diag, deg [NROW]."""
    bm = np.zeros((NROW, NROW), np.float32)
    for b in range(BC):
        d = np.linalg.norm(coor[b][:, None, :] - coor[b][None, :, :], axis=-1)
        m = (d <= (r[b][:, None] + r[b][None, :])) & ~np.eye(N, dtype=bool)
        bm[b * N:(b + 1) * N, b * N:(b + 1) * N] = m
    return bm, bm.sum(1)


def internet_host(s, bm, deg, p):
    """s [NROW, D] fp32 row-major; bm [NROW,NROW]; p = (sw,sb,rw,rb,aw,ab,ow,ob)."""
    sw, sb, rw, rb, aw, ab, ow, ob = p
    Wl, Wr = rw[:, :D], rw[:, D:]
    self_d = s @ sw.T + sb
    u = s @ Wl.T + rb
    v = s @ Wr.T
    rel = deg[:, None] * u + bm @ v
    a = np.maximum((self_d + rel) @ aw.T + ab, 0)
    return np.maximum(a @ ow[:, :D].T + s @ ow[:, D:].T + ob, 0)


def gnn_host(obj_t, src_coor, r, inputs):
    """obj_t [4][NROW, D] initial states; src_coor [BC, T, N, 2]; r [BC, N].
    Returns bboxes [BC, 8, N, 4]."""
    states = list(obj_t)
    masks = [mask_host(src_coor[:, t], r) for t in range(4)]
    num_rollouts = int(inputs['num_rollouts'])
    out = []
    for rr in range(num_rollouts):
        cs = []
        for k in range(4):
            p = (inputs['g_self_w'][k], inputs['g_self_b'][k], inputs['g_rel_w'][k],
                 inputs['g_rel_b'][k], inputs['g_aff_w'][k], inputs['g_aff_b'][k],
                 inputs['g_out_w'][k], inputs['g_out_b'][k])
            bm, deg = masks[k]
            cs.append(internet_host(states[k], bm, deg, p))
        s = np.concatenate(cs, -1) @ inputs['agg_w'].T + inputs['agg_b']
        bbox = s @ inputs['dec_w'].T + inputs['dec_b']          # [NROW, 4]
        out.append(bbox.reshape(BC, N, 4))
        states = states[1:] + [s]
        coor = bbox[:, 2:].reshape(BC, N, 2)
        masks = masks[1:] + [mask_host(coor, r)]
    return np.stack(out, 1)


def full_host(inputs, shard):
    """Complete per-core mirror (fp32). shard = B-slice index."""
    sl = slice(shard * BC, (shard + 1) * BC)
    x = inputs['x'][sl].reshape(NIMG, CIN, IMG, IMG)
    rois = inputs['rois'][sl].reshape(NROI, 5)
    coor = inputs['src_coor_features'][sl]                      # [BC, T, N, 2]
    r = (((rois.reshape(BC, T, N, 5)[..., 4] - rois.reshape(BC, T, N, 5)[..., 2]) / 2
          + (rois.reshape(BC, T, N, 5)[..., 3] - rois.reshape(BC, T, N, 5)[..., 1]) / 2) / 2).mean(1)
    f1 = np.maximum(conv1_host(x, inputs['w_conv1'], inputs['b_conv1']), 0)
    f2 = np.maximum(conv2_host(f1, inputs['w_conv2'], inputs['b_conv2']), 0)
    pooled = roi_align_host(f2, rois)                           # [NPT, 64] pt-major
    # fc0: obj[row, d] = sum_{c,pt} pool[row, pt, c] * fc0_w[d, c*16+pt]
    pool_cp = pooled.reshape(NROI, 16, 64)
    Wp = inputs['fc0_w'].reshape(D, 64, 16)                     # [d, c, pt]
    obj = np.einsum('rpc,dcp->rd', pool_cp, Wp) + inputs['fc0_b']
    obj = np.maximum(obj, 0)                                    # [NROI, D] rows (b,t,n)
    emb = np.maximum(coor.reshape(NROI, 2) @ inputs['fc0c_w'].T + inputs['fc0c_b'], 0)
    emb = np.maximum(emb @ inputs['fc1c_w'].T + inputs['fc1c_b'], 0)
    o2 = np.maximum(obj @ inputs['red_w'][:, :D].T + emb @ inputs['red_w'][:, D:].T
                    + inputs['red_b'], 0)                       # [NROI, D]
    o2 = o2.reshape(BC, T, N, D)
    obj_t = [o2[:, t].reshape(NROW, D) for t in range(4)]
    return gnn_host(obj_t, coor, r, inputs)


# ---------------- device input packing ----------------
def make_core_inputs(inputs, shard):
    import ml_dtypes
    bf16 = ml_dtypes.bfloat16
    sl = slice(shard * BC, (shard + 1) * BC)
    x = np.asarray(inputs['x'][sl], np.float32).reshape(NIMG, CIN, IMG, IMG)
    rois = np.asarray(inputs['rois'][sl], np.float32).reshape(NROI, 5)
    coor = np.asarray(inputs['src_coor_features'][sl], np.float32)   # [BC,T,N,2]
    rr5 = rois.reshape(BC, T, N, 5)
    r = (((rr5[..., 4] - rr5[..., 2]) / 2 + (rr5[..., 3] - rr5[..., 1]) / 2) / 2).mean(1)

    fp8 = ml_dtypes.float8_e4m3
    cdt = fp8 if FP8_CONV else bf16
    d = {}
    cols = conv1_im2col_host(x).reshape(45, -1)       # [45, NIMG*64*32]
    if FP8_CONV:
        c46 = np.zeros((46, cols.shape[1]), np.float32)
        c46[:45] = cols
        # DoubleRow pair layout: row p holds tap p | tap 23+p side by side
        d['im2col45'] = np.concatenate([c46[:23], c46[23:]], 1).astype(fp8)
        w46 = np.zeros((46, 128), np.float32)
        w46[:45] = conv1_weights_host(np.asarray(inputs['w_conv1']))
        d['w1'] = np.concatenate([w46[:23], w46[23:]], 1).astype(fp8)
    else:
        d['im2col45'] = cols.astype(bf16)
        d['w1'] = conv1_weights_host(np.asarray(inputs['w_conv1'])).astype(bf16)
    b1 = np.asarray(inputs['b_conv1'], np.float32)
    d['b1'] = np.tile(b1, 2).reshape(128, 1).astype(np.float32)
    Wb = conv2_weights_host(np.asarray(inputs['w_conv2']))
    d['w2a'] = np.stack([Wb[dy][0] for dy in range(3)]).astype(cdt)
    d['w2b'] = np.stack([Wb[dy][1] for dy in range(3)]).astype(cdt)
    d['w2c'] = np.stack([Wb[dy][2] for dy in range(3)]).astype(cdt)
    b2 = np.asarray(inputs['b_conv2'], np.float32)
    d['b2'] = np.tile(b2, 2).reshape(128, 1).astype(np.float32)

    Wg = roi_wmat_host(rois)                          # [NIMG, 8, 128, 96]
    # device layout per group g: [128 pair_local, (img_local, chunk, pt) 6144]
    d['wroi'] = np.ascontiguousarray(
        Wg.reshape(NGRP, IMG_GRP, 8, 128, 96).transpose(0, 3, 1, 2, 4)
        .reshape(NGRP, 128, IMG_GRP * 8 * 96)).astype(bf16)

    fc0w = np.asarray(inputs['fc0_w'], np.float32).reshape(D, 64, 16)  # [d, c, pt]
    d['fc0t'] = np.ascontiguousarray(fc0w.transpose(2, 1, 0)).astype(bf16)  # [pt, c, d]
    d['fc0b'] = np.asarray(inputs['fc0_b'], np.float32).reshape(2, 128).T.copy()

    d['coor_fm'] = coor.reshape(NROI, 2).T.astype(bf16).copy()

    def t2(w):   # [256, K] -> [kc, 128, 256] lhsT chunks (w.T row-chunks)
        wT = np.ascontiguousarray(np.asarray(w, np.float32).T)       # [K, 256]
        K = wT.shape[0]
        return wT.reshape(K // 128, 128, 256).astype(bf16)

    def bcol(b):  # [256] -> [128, 2]
        return np.asarray(b, np.float32).reshape(2, 128).T.copy()

    d['fc0ct'] = np.asarray(inputs['fc0c_w'], np.float32).T.astype(bf16).copy()  # [2, 256]
    d['fc0cb'] = bcol(inputs['fc0c_b'])
    d['fc1ct'] = t2(inputs['fc1c_w'])
    d['fc1cb'] = bcol(inputs['fc1c_b'])
    redw = np.asarray(inputs['red_w'], np.float32)
    d['redoT'] = t2(redw[:, :D])
    d['redeT'] = t2(redw[:, D:])
    d['redb'] = bcol(inputs['red_b'])

    d['gswT'] = np.stack([t2(inputs['g_self_w'][k]) for k in range(4)])
    grw = np.asarray(inputs['g_rel_w'], np.float32)
    d['gWlT'] = np.stack([t2(grw[k][:, :D]) for k in range(4)])
    d['gWrT'] = np.stack([t2(grw[k][:, D:]) for k in range(4)])
    d['gawT'] = np.stack([t2(inputs['g_aff_w'][k]) for k in range(4)])
    gow = np.asarray(inputs['g_out_w'], np.float32)
    d['gowaT'] = np.stack([t2(gow[k][:, :D]) for k in range(4)])
    d['gowsT'] = np.stack([t2(gow[k][:, D:]) for k in range(4)])
    d['gbiasT'] = np.concatenate([
        np.asarray(inputs['g_self_b'], np.float32).reshape(-1),
        np.asarray(inputs['g_aff_b'], np.float32).reshape(-1),
        np.asarray(inputs['g_out_b'], np.float32).reshape(-1)]).reshape(1, 3072).astype(bf16)
    d['aggT'] = t2(inputs['agg_w'])                    # [8, 128, 256]
    d['aggbT'] = np.asarray(inputs['agg_b'], np.float32).reshape(1, 256).astype(bf16)
    decw = np.asarray(inputs['dec_w'], np.float32)     # [4, 256]
    d['decT'] = decw.T.reshape(2, 128, 4).astype(bf16).copy()
    d['decbT'] = np.asarray(inputs['dec_b'], np.float32).reshape(1, 4).astype(bf16)

    hmdds = []
    for m in range(4):
        bm, deg = mask_host(coor[:, m], r)
        mdd = np.zeros((112, NROW), np.float32)
        mdd[0:48] = np.diag(deg)
        mdd[64:112] = bm
        hmdds.append(mdd.astype(bf16))
    d['hmdd'] = np.stack(hmdds)
    d['rbT'] = np.asarray(inputs['g_rel_b'], np.float32).reshape(1, 1024).astype(bf16)
    d['zrow'] = np.zeros((1, 256), bf16)
    Tmat = np.full((NROW, NROW), -1.0, np.float32)
    for b in range(BC):
        rs = (r[b][:, None] + r[b][None, :]) ** 2
        np.fill_diagonal(rs, -1.0)
        Tmat[b * N:(b + 1) * N, b * N:(b + 1) * N] = rs
    Tm112 = np.zeros((112, NROW), np.float32)
    Tm112[64:112] = Tmat
    d['Tm'] = Tm112
    d['ones48'] = np.ones((112, 128), bf16)
    d['ones2'] = np.ones((2, 48), bf16)
    d['ident'] = np.eye(128, dtype=bf16)
    d['eye48'] = np.eye(48, dtype=np.float32)
    return d


dt = mybir.dt
AF = mybir.ActivationFunctionType
OP = mybir.AluOpType

NIMG, NROI, NROW, NPT = 32, 192, 48, 3072
NG = 24            # gather groups (128 pts each)
IMG_GRP = 8        # images per conv group
NGRP = NIMG // IMG_GRP
IMGF = 2 * 33 * 33  # 2178 free els per img in feat1_ph


def build(nc: bass.Bass, dump=False, stage='full', nrep=1):
    f32, bf16, i32 = dt.float32, dt.bfloat16, dt.int32
    f8 = dt.float8e4 if FP8_CONV else dt.bfloat16

    def din(name, shape, d):
        return nc.dram_tensor(name, shape, d, kind="ExternalInput")

    if FP8_CONV:
        im2col = din("im2col45", [23, 131072], f8)
        w1 = din("w1", [23, 256], f8)
    else:
        im2col = din("im2col45", [45, 65536], f8)
        w1 = din("w1", [45, 128], f8)
    b1 = din("b1", [128, 1], f32)
    w2a = din("w2a", [3, 128, 128], f8)
    w2b = din("w2b", [3, 128, 128], f8)
    w2c = din("w2c", [3, 64, 128], f8)
    b2 = din("b2", [128, 1], f32)
    wroi = din("wroi", [NGRP, 128, IMG_GRP * 8 * 96], bf16)
    fc0t = din("fc0t", [16, 64, 256], bf16)
    fc0b = din("fc0b", [128, 2], f32)
    coor = din("coor_fm", [2, 192], bf16)
    fc0ct = din("fc0ct", [2, 256], bf16)
    fc0cb = din("fc0cb", [128, 2], f32)
    fc1ct = din("fc1ct", [2, 128, 256], bf16)
    fc1cb = din("fc1cb", [128, 2], f32)
    redoT = din("redoT", [2, 128, 256], bf16)
    redeT = din("redeT", [2, 128, 256], bf16)
    redb = din("redb", [128, 2], f32)
    gswT = din("gswT", [4, 2, 128, 256], bf16)
    gWlT = din("gWlT", [4, 2, 128, 256], bf16)
    gWrT = din("gWrT", [4, 2, 128, 256], bf16)
    gawT = din("gawT", [4, 2, 128, 256], bf16)
    gowaT = din("gowaT", [4, 2, 128, 256], bf16)
    gowsT = din("gowsT", [4, 2, 128, 256], bf16)
    gbiasT = din("gbiasT", [1, 3072], bf16)
    rbT = din("rbT", [1, 1024], bf16)
    zrow = din("zrow", [1, 256], bf16)
    aggT = din("aggT", [8, 128, 256], bf16)
    aggbT = din("aggbT", [1, 256], bf16)
    decT = din("decT", [2, 128, 4], bf16)
    decbT = din("decbT", [1, 4], bf16)
    hmdd = din("hmdd", [4, 112, 48], bf16)
    Tm = din("Tm", [112, 48], f32)
    ones48 = din("ones48", [112, 128], bf16)
    ones2 = din("ones2", [2, 48], bf16)
    ident = din("ident", [128, 128], bf16)
    eye48 = din("eye48", [48, 48], f32)

    out = nc.dram_tensor("bbox_out", [8, 8, 6, 4], f32, kind="ExternalOutput")
    if dump:
        dbg_mdd = nc.dram_tensor("dbg_mdd", [112, 192], bf16, kind="ExternalOutput")
        dbg_uvt = nc.dram_tensor("dbg_uvt", [112, 1024], bf16, kind="ExternalOutput")
        dbg_cs = nc.dram_tensor("dbg_cs", [128, 384], bf16, kind="ExternalOutput")
        dbg_st = nc.dram_tensor("dbg_st", [128, 480], bf16, kind="ExternalOutput")
        dbg_x = nc.dram_tensor("dbg_x", [128, 384], bf16, kind="ExternalOutput")
        dbg_a = nc.dram_tensor("dbg_a", [128, 384], bf16, kind="ExternalOutput")

    with tile.TileContext(nc) as tc, ExitStack() as ctx:
        # ---- persistent pools ----
        wp = ctx.enter_context(tc.tile_pool(name="w", bufs=1))
        sp = ctx.enter_context(tc.tile_pool(name="state", bufs=1))

        def load(dram_t, shape, dtype, src_ap=None):
            t = wp.tile(shape, dtype, tag=dram_t.name)
            if src_ap is None:
                nc.sync.dma_start(t[:], dram_t[:, :])
            else:
                # src_ap dims [p, d0, d1, ...]; dst = t reshaped to match
                dims = [c for _, c in src_ap.ap[1:]]
                spec = " ".join(f"d{i}" for i in range(len(dims)))
                kw = {f"d{i}": dims[i] for i in range(len(dims) - 1)}
                dv = t[:].rearrange(f"p ({spec}) -> p {spec}", **kw)
                nc.sync.dma_start(dv, src_ap)
            return t

        # conv-critical loads first so im2col g0 isn't queued behind ~4MB of
        # GNN weights; everything else loads mid-body, overlapped with conv.
        w1_s = load(w1, [23, 256] if FP8_CONV else [45, 128], f8)
        b1_s = load(b1, [128, 1], f32)
        w2a_s = load(w2a, [128, 3 * 128], f8, w2a[:].rearrange("d p m -> p d m"))
        w2b_s = load(w2b, [128, 3 * 128], f8, w2b[:].rearrange("d p m -> p d m"))
        w2c_s = load(w2c, [64, 3 * 128], f8, w2c[:].rearrange("d p m -> p d m"))
        b2_s = load(b2, [128, 1], f32)
        ident_s = load(ident, [128, 128], bf16)

        class _LW: pass
        lw = _LW()

        def late_loads():
            lw.fc0t_s = load(fc0t, [64, 16 * 256], bf16,
                                  fc0t[:].rearrange("t p m -> p t m"))
            lw.fc0b_s = load(fc0b, [128, 2], f32)
            lw.coor_s = load(coor, [2, 192], bf16)
            lw.fc0ct_s = load(fc0ct, [2, 256], bf16)
            lw.fc0cb_s = load(fc0cb, [128, 2], f32)
            lw.fc1ct_s = load(fc1ct, [128, 512], bf16,
                                   fc1ct[:].rearrange("k p m -> p k m"))
            lw.fc1cb_s = load(fc1cb, [128, 2], f32)
            lw.redoT_s = load(redoT, [128, 512], bf16,
                                   redoT[:].rearrange("k p m -> p k m"))
            lw.redeT_s = load(redeT, [128, 512], bf16,
                                   redeT[:].rearrange("k p m -> p k m"))
            lw.redb_s = load(redb, [128, 2], f32)

            def loadg(t):  # [4,2,128,256] -> [128, 4*512]
                return load(t, [128, 2048], bf16, t[:].rearrange("h k p m -> p h k m"))
            lw.gswT_s, lw.gWlT_s, lw.gWrT_s = loadg(gswT), loadg(gWlT), loadg(gWrT)
            lw.gawT_s, lw.gowaT_s, lw.gowsT_s = loadg(gawT), loadg(gowaT), loadg(gowsT)
            lw.gbiasT_s = load(gbiasT, [1, 3072], bf16)
            lw.aggT_s = load(aggT, [128, 2048], bf16,
                                  aggT[:].rearrange("k p m -> p k m"))
            lw.aggbT_s = load(aggbT, [1, 256], bf16)
            lw.decT_s = load(decT, [128, 8], bf16,
                                  decT[:].rearrange("k p m -> p k m"))
            lw.decbT_s = load(decbT, [1, 4], bf16)
            lw.Tm_s = load(Tm, [112, 48], f32)
            lw.ones48_s = load(ones48, [112, 128], bf16)
            lw.ones2_s = load(ones2, [2, 48], bf16)
            lw.eye48_s = load(eye48, [48, 48], f32)
            lw.rbT_s = load(rbT, [1, 1024], bf16)
            lw.zrow_s = load(zrow, [1, 256], bf16)
            for m in range(4):
                nc.sync.dma_start(mdd_t[m][:], hmdd[m])
            for m in range(4, 11):
                nc.vector.memset(mdd_t[m][32:64, :], 0.0)


        # mask/ddiag slots [112,48]: rows 0-47 diag(deg), 48-63 zero,
        # 64-111 mask (engine writes must start at partition 0/32/64/96)
        mdd_t = [sp.tile([112, 48], bf16, name=f"mdd{m}", tag=f"mdd{m}") for m in range(11)]
        # per-head [u+rb; 0; v] lhsT tiles (rel bias folded in via K=1 matmul)
        uvt = [sp.tile([112, 256], bf16, name=f"uvt{k}", tag=f"uvt{k}") for k in range(4)]

        st = [sp.tile([128, 96], bf16, name=f"st{m}", tag=f"st{m}") for m in range(12)]
        bbox_sb = sp.tile([4, 384], f32, tag="bbox")
        poolT = sp.tile([64, 3072], bf16, tag="poolT")

        def stages():
            if stage == 'setup':
                return

            # ================= conv stage =================
            with ExitStack() as cvx:
                imcp = cvx.enter_context(tc.tile_pool(name="imc", bufs=2))
                f1p = cvx.enter_context(tc.tile_pool(name="f1", bufs=2))
                c1ps = cvx.enter_context(tc.tile_pool(name="c1ps", bufs=2, space="PSUM"))
                c2ps = cvx.enter_context(tc.tile_pool(name="c2ps", bufs=2, space="PSUM"))
                tps = cvx.enter_context(tc.tile_pool(name="tps", bufs=1, space="PSUM"))
                pps = cvx.enter_context(tc.tile_pool(name="pps", bufs=1, space="PSUM"))
                f2p = cvx.enter_context(tc.tile_pool(name="f2", bufs=3))
                wrp = cvx.enter_context(tc.tile_pool(name="wr", bufs=2))

                GC = IMG_GRP * 2048
                for g in range(NGRP):
                    if FP8_CONV:
                        imc = imcp.tile([23, 2 * IMG_GRP * 2048], f8, tag="imc")
                        imv = imc[:].rearrange("p (i n) -> p i n", i=2)
                        nc.sync.dma_start(imv[:, 0, :], im2col[:, g * GC:(g + 1) * GC])
                        nc.sync.dma_start(imv[:, 1, :],
                                          im2col[:, 65536 + g * GC:65536 + (g + 1) * GC])
                    else:
                        imc = imcp.tile([45, IMG_GRP * 2048], f8, tag="imc")
                        nc.sync.dma_start(imc[:], im2col[:, g * GC:(g + 1) * GC])
                    f1 = f1p.tile([128, IMG_GRP * IMGF], f8, tag="f1")
                    # zero halo strips (Y=32 row, X=32 col)
                    f1v = f1[:].rearrange("p (i y x) -> p i y x", i=IMG_GRP, y=2 * 33, x=33)
                    nc.gpsimd.memset(f1v[:, :, :, 32:33], 0.0)
                    f1h = f1[:].rearrange("p (i py y x) -> p i py y x", i=IMG_GRP, py=2, y=33, x=33)
                    nc.gpsimd.memset(f1h[:, :, :, 32:33, :], 0.0)
                    for i in range(IMG_GRP):
                        # conv1: 4 matmuls of [45,128]x[45,512] -> psum [128,1024] x2
                        pv = []
                        for h in range(2):
                            ps = c1ps.tile([128, 1024], f32, tag="c1")
                            for q in range(2):
                                off = i * 2048 + h * 1024 + q * 512
                                if FP8_CONV:
                                    rhs = bass.AP(imc[:].tensor, off,
                                                  [imc[:].ap[0], [16384, 2], [1, 512]])
                                    nc.tensor.matmul(ps[:, q * 512:(q + 1) * 512],
                                                     lhsT=w1_s[:].rearrange(
                                                         "p (i m) -> p i m", i=2),
                                                     rhs=rhs, start=True, stop=True,
                                                     perf_mode=mybir.MatmulPerfMode.DoubleRow)
                                else:
                                    nc.tensor.matmul(ps[:, q * 512:(q + 1) * 512],
                                                     lhsT=w1_s[:],
                                                     rhs=imc[:, off:off + 512],
                                                     start=True, stop=True)
                            pv.append(ps)
                        # evac relu+bias: one 3-dim-AP op per half covers both
                        # py phases (psum cols (y32, j32) -> f1 (py, yo, x))
                        for h in range(2):
                            ps_t = pv[h][:]
                            src = bass.AP(ps_t.tensor, ps_t.offset,
                                          [ps_t.ap[0], [32, 2], [64, 16], [1, 32]])
                            dst = f1h[:, i, :, 16 * h:16 * h + 16, 0:32]
                            if h == 0:
                                nc.vector.tensor_scalar(
                                    out=dst, in0=src, scalar1=b1_s[:, 0:1],
                                    scalar2=0.0, op0=OP.add, op1=OP.max)
                            else:
                                nc.scalar.activation(out=dst, in_=src,
                                                     func=AF.Relu, bias=b1_s[:, 0:1])
                    for i in range(IMG_GRP):
                        # conv2: 9 matmuls -> psum [128, 512] cols (oy2 32, j2 16)
                        wr = wrp.tile([128, 8 * 96], bf16, tag="wr")
                        nc.sync.dma_start(wr[:], wroi[g][:, i * 768:(i + 1) * 768])
                        ps = c2ps.tile([128, 512], f32, tag="c2")
                        first = True
                        f1v5 = f1[:].rearrange("p (i py y x) -> p i py y x",
                                               i=IMG_GRP, py=2, y=33, x=33)
                        for dy in range(3):
                            py, yo = dy % 2, dy // 2
                            for gsel in range(3):
                                sl = f1v5[:, i, py, yo:yo + 32, gsel:gsel + 1]
                                rhs_ap = bass.AP(sl.tensor, sl.offset,
                                                 [sl.ap[0], sl.ap[1], [2, 16]])
                                if gsel == 2:
                                    rhs_ap = rhs_ap[0:64]
                                    lhsT = w2c_s[:, dy * 128:(dy + 1) * 128]
                                else:
                                    lhsT = (w2a_s if gsel == 0 else w2b_s)[:, dy * 128:(dy + 1) * 128]
                                nc.tensor.matmul(ps[:], lhsT=lhsT, rhs=rhs_ap,
                                                 start=first, stop=(dy == 2 and gsel == 2))
                                first = False
                        f2s = f2p.tile([128, 512], bf16, tag="f2s")
                        if i % 2 == 0:
                            nc.vector.tensor_scalar(out=f2s[:], in0=ps[:], scalar1=b2_s[:, 0:1],
                                                    scalar2=0.0, op0=OP.add, op1=OP.max)
                        else:
                            nc.scalar.activation(out=f2s[:], in_=ps[:], func=AF.Relu,
                                                 bias=b2_s[:, 0:1])
                        tp = tps.tile([128, 512], bf16, tag="tp")
                        for b in range(4):
                            nc.tensor.transpose(tp[:, b * 128:(b + 1) * 128],
                                                f2s[:, b * 128:(b + 1) * 128], ident_s[:])
                        f2t = f2p.tile([128, 512], bf16, tag="f2t")
                        if i % 2 == 0:
                            nc.scalar.activation(out=f2t[:], in_=tp[:], func=AF.Copy)
                        else:
                            nc.vector.tensor_copy(out=f2t[:], in_=tp[:])
                        # RoIAlign as matmul: pool_ps[c, n*16+pt] = sum over
                        # pixel chunks (b, px) of f2t-slice^T @ wroi-slice
                        img = g * IMG_GRP + i
                        pool_ps = pps.tile([64, 96], f32, tag="pool")
                        for c in range(8):
                            b, px = divmod(c, 2)
                            nc.tensor.matmul(
                                pool_ps[:],
                                lhsT=f2t[:, b * 128 + px * 64:b * 128 + px * 64 + 64],
                                rhs=wr[:, c * 96:c * 96 + 96],
                                start=(c == 0), stop=(c == 7))
                        if i % 2 == 0:
                            nc.scalar.activation(out=poolT[:, img * 96:(img + 1) * 96],
                                                 in_=pool_ps[:], func=AF.Copy)
                        else:
                            nc.vector.tensor_copy(out=poolT[:, img * 96:(img + 1) * 96],
                                                  in_=pool_ps[:])

            if not getattr(lw, 'done', False):
                lw.done = True
                late_loads()
            if stage == 'conv':
                return

            # ================= fc0 + emb + red =================
            with ExitStack() as gx:
                ops = gx.enter_context(tc.tile_pool(name="ops", bufs=2, space="PSUM"))

                obj = sp.tile([128, 384], bf16, tag="obj")
                pview = poolT[:].rearrange("p (r t) -> p t r", t=16)
                for m2 in range(2):
                    ps = ops.tile([128, 192], f32, tag="obj")
                    for pt_i in range(16):
                        nc.tensor.matmul(ps[:], lhsT=lw.fc0t_s[:, pt_i * 256 + m2 * 128:
                                                            pt_i * 256 + m2 * 128 + 128],
                                         rhs=pview[:, pt_i, :],
                                         start=(pt_i == 0), stop=(pt_i == 15))
                    nc.scalar.activation(out=obj[:, m2 * 192:(m2 + 1) * 192], in_=ps[:],
                                         func=AF.Relu, bias=lw.fc0b_s[:, m2:m2 + 1])
                emb1 = sp.tile([128, 384], bf16, tag="emb1")
                for m2 in range(2):
                    ps = ops.tile([128, 192], f32, tag="emb")
                    nc.tensor.matmul(ps[:], lhsT=lw.fc0ct_s[:, m2 * 128:(m2 + 1) * 128],
                                     rhs=lw.coor_s[:], start=True, stop=True)
                    nc.scalar.activation(out=emb1[:, m2 * 192:(m2 + 1) * 192], in_=ps[:],
                                         func=AF.Relu, bias=lw.fc0cb_s[:, m2:m2 + 1])
                emb2 = sp.tile([128, 384], bf16, tag="emb2")
                for m2 in range(2):
                    ps = ops.tile([128, 192], f32, tag="emb")
                    for kc in range(2):
                        nc.tensor.matmul(ps[:], lhsT=lw.fc1ct_s[:, kc * 256 + m2 * 128:
                                                             kc * 256 + m2 * 128 + 128],
                                         rhs=emb1[:, kc * 192:(kc + 1) * 192],
                                         start=(kc == 0), stop=(kc == 1))
                    nc.scalar.activation(out=emb2[:, m2 * 192:(m2 + 1) * 192], in_=ps[:],
                                         func=AF.Relu, bias=lw.fc1cb_s[:, m2:m2 + 1])
                o2 = sp.tile([128, 384], bf16, tag="o2")
                for m2 in range(2):
                    ps = ops.tile([128, 192], f32, tag="o2")
                    for kc in range(2):
                        nc.tensor.matmul(ps[:], lhsT=lw.redoT_s[:, kc * 256 + m2 * 128:
                                                             kc * 256 + m2 * 128 + 128],
                                         rhs=obj[:, kc * 192:(kc + 1) * 192],
                                         start=(kc == 0), stop=False)
                    for kc in range(2):
                        nc.tensor.matmul(ps[:], lhsT=lw.redeT_s[:, kc * 256 + m2 * 128:
                                                             kc * 256 + m2 * 128 + 128],
                                         rhs=emb2[:, kc * 192:(kc + 1) * 192],
                                         start=False, stop=(kc == 1))
                    nc.scalar.activation(out=o2[:, m2 * 192:(m2 + 1) * 192], in_=ps[:],
                                         func=AF.Relu, bias=lw.redb_s[:, m2:m2 + 1])
                # initial states: s_m [128, 96] cols m2*48 + b*6 + n  <- o2 cols m2*192 + b*24 + m*6 + n
                o2v = o2[:].rearrange("p (m2 b t n) -> p m2 b t n", m2=2, b=8, t=4)
                for m in range(4):
                    nc.vector.tensor_copy(
                        out=st[m][:].rearrange("p (m2 b n) -> p m2 b n", m2=2, b=8),
                        in_=o2v[:, :, :, m, :])

            if stage.startswith('gather'):
                return

            # ================= GNN rollouts =================
            with ExitStack() as rx:
                gps = rx.enter_context(tc.tile_pool(name="gps", bufs=4, space="PSUM"))
                vps = rx.enter_context(tc.tile_pool(name="vps", bufs=2, space="PSUM"))
                sps = rx.enter_context(tc.tile_pool(name="sps", bufs=2, space="PSUM"))
                hb = rx.enter_context(tc.tile_pool(name="hbuf", bufs=3))

                def emit_uv(rr, ks):
                    # uv_ps rows 0-47 = u+rb = s@Wl^T + rb (rb via K=1 matmul),
                    # rows 64-111 = v = s@Wr^T; contiguous accumulation group
                    # per partition region; copies alternate DVE/Act
                    for k in ks:
                        s = st[rr + k]
                        uv_ps = vps.tile([112, 256], f32, tag="v")
                        # zero rows 32-63 first (write base must be 0/32/64; the
                        # u matmuls below re-cover 32-47 with real data)
                        nc.tensor.matmul(uv_ps[32:64, :], lhsT=lw.ones2_s[0:1, 0:32],
                                         rhs=lw.zrow_s[:], start=True, stop=True)
                        for kc in range(2):
                            nc.tensor.matmul(uv_ps[0:48, :], lhsT=s[:, kc * 48:kc * 48 + 48],
                                             rhs=lw.gWlT_s[:, k * 512 + kc * 256:
                                                        k * 512 + (kc + 1) * 256],
                                             start=(kc == 0), stop=False)
                        nc.tensor.matmul(uv_ps[0:48, :], lhsT=lw.ones2_s[0:1, :],
                                         rhs=lw.rbT_s[:, k * 256:(k + 1) * 256],
                                         start=False, stop=True)
                        for kc in range(2):
                            nc.tensor.matmul(uv_ps[64:112, :], lhsT=s[:, kc * 48:kc * 48 + 48],
                                             rhs=lw.gWrT_s[:, k * 512 + kc * 256:
                                                        k * 512 + (kc + 1) * 256],
                                             start=(kc == 0), stop=(kc == 1))
                        if k % 2 == 0:
                            nc.vector.tensor_copy(out=uvt[k][:], in_=uv_ps[:])
                        else:
                            nc.scalar.activation(out=uvt[k][:], in_=uv_ps[:], func=AF.Copy)

                emit_uv(0, range(4))
                for rr in range(8):
                    # stage-major emission: all 4 heads per stage so PE never
                    # head-of-line blocks on one head's evacuations; uv for
                    # rollout rr+1 is software-pipelined into rr's tail
                    x_pss, x_sbs, a_pss, a_sbs, o_pss, cs = [], [], [], [], [], []
                    for k in range(4):
                        m = rr + k
                        s = st[rr + k]
                        # x = rel + deg*(u+rb) + self-dynamics; one contiguous psum
                        # accumulation group per half (interleaved groups in one
                        # bank mis-accumulate): rel first, then sd matmuls
                        x_ps = gps.tile([128, 96], f32, tag="g")
                        for m2 in range(2):
                            nc.tensor.matmul(x_ps[:, m2 * 48:m2 * 48 + 48],
                                             lhsT=uvt[k][:, m2 * 128:(m2 + 1) * 128],
                                             rhs=mdd_t[m][:], start=True, stop=False)
                            for kc in range(2):
                                lo = k * 512 + kc * 256 + m2 * 128
                                nc.tensor.matmul(x_ps[:, m2 * 48:m2 * 48 + 48],
                                                 lhsT=lw.gswT_s[:, lo:lo + 128],
                                                 rhs=s[:, kc * 48:kc * 48 + 48],
                                                 start=False, stop=False)
                            nc.tensor.matmul(x_ps[:, m2 * 48:m2 * 48 + 48],
                                             lhsT=lw.gbiasT_s[:, k * 256 + m2 * 128:
                                                              k * 256 + m2 * 128 + 128],
                                             rhs=lw.ones2_s[0:1, :],
                                             start=False, stop=True)
                        x_pss.append(x_ps)
                    for k in range(4):
                        x_sb = hb.tile([128, 96], bf16, tag=f"x{k}")
                        if k % 2 == 0:
                            nc.vector.tensor_copy(out=x_sb[:], in_=x_pss[k][:])
                        else:
                            nc.scalar.activation(out=x_sb[:], in_=x_pss[k][:], func=AF.Copy)
                        x_sbs.append(x_sb)
                    for k in range(4):
                        a_ps = gps.tile([128, 96], f32, tag="g")
                        for m2 in range(2):
                            for kc in range(2):
                                lo = k * 512 + kc * 256 + m2 * 128
                                nc.tensor.matmul(a_ps[:, m2 * 48:m2 * 48 + 48],
                                                 lhsT=lw.gawT_s[:, lo:lo + 128],
                                                 rhs=x_sbs[k][:, kc * 48:kc * 48 + 48],
                                                 start=(kc == 0), stop=False)
                            nc.tensor.matmul(a_ps[:, m2 * 48:m2 * 48 + 48],
                                             lhsT=lw.gbiasT_s[:, 1024 + k * 256 + m2 * 128:
                                                              1024 + k * 256 + m2 * 128 + 128],
                                             rhs=lw.ones2_s[0:1, :],
                                             start=False, stop=True)
                        a_pss.append(a_ps)
                    for k in range(4):
                        a_sb = hb.tile([128, 96], bf16, tag=f"a{k}")
                        if k % 2 == 0:
                            nc.scalar.activation(out=a_sb[:], in_=a_pss[k][:],
                                                 func=AF.Relu, bias=0.0)
                        else:
                            nc.vector.tensor_scalar(out=a_sb[:], in0=a_pss[k][:],
                                                    scalar1=0.0, scalar2=None, op0=OP.max)
                        a_sbs.append(a_sb)
                    for k in range(4):
                        s = st[rr + k]
                        o_ps = gps.tile([128, 96], f32, tag="g")
                        for m2 in range(2):
                            for kc in range(2):
                                lo = k * 512 + kc * 256 + m2 * 128
                                nc.tensor.matmul(o_ps[:, m2 * 48:m2 * 48 + 48],
                                                 lhsT=lw.gowaT_s[:, lo:lo + 128],
                                                 rhs=a_sbs[k][:, kc * 48:kc * 48 + 48],
                                                 start=(kc == 0), stop=False)
                                nc.tensor.matmul(o_ps[:, m2 * 48:m2 * 48 + 48],
                                                 lhsT=lw.gowsT_s[:, lo:lo + 128],
                                                 rhs=s[:, kc * 48:kc * 48 + 48],
                                                 start=False, stop=False)
                            nc.tensor.matmul(o_ps[:, m2 * 48:m2 * 48 + 48],
                                             lhsT=lw.gbiasT_s[:, 2048 + k * 256 + m2 * 128:
                                                              2048 + k * 256 + m2 * 128 + 128],
                                             rhs=lw.ones2_s[0:1, :],
                                             start=False, stop=True)
                        o_pss.append(o_ps)
                    for k in range(4):
                        c_sb = hb.tile([128, 96], bf16, tag=f"cs{k}")
                        if k % 2 == 0:
                            nc.scalar.activation(out=c_sb[:], in_=o_pss[k][:],
                                                 func=AF.Relu, bias=0.0)
                        else:
                            nc.vector.tensor_scalar(out=c_sb[:], in0=o_pss[k][:],
                                                    scalar1=0.0, scalar2=None, op0=OP.max)
                        cs.append(c_sb)
                    if dump and rr == 0:
                        for k in range(4):
                            nc.sync.dma_start(dbg_uvt[:, k * 256:(k + 1) * 256], uvt[k][:])
                            nc.sync.dma_start(dbg_x[:, k * 96:(k + 1) * 96], x_sbs[k][:])
                            nc.sync.dma_start(dbg_a[:, k * 96:(k + 1) * 96], a_sbs[k][:])
                    if rr < 7:
                        emit_uv(rr + 1, range(3))
                    g_ps = gps.tile([128, 96], f32, tag="g")
                    for m2 in range(2):
                        n = 0
                        for k in range(4):
                            for kc in range(2):
                                lo = (k * 2 + kc) * 256 + m2 * 128
                                nc.tensor.matmul(g_ps[:, m2 * 48:m2 * 48 + 48],
                                                 lhsT=lw.aggT_s[:, lo:lo + 128],
                                                 rhs=cs[k][:, kc * 48:kc * 48 + 48],
                                                 start=(n == 0), stop=False)
                                n += 1
                        nc.tensor.matmul(g_ps[:, m2 * 48:m2 * 48 + 48],
                                         lhsT=lw.aggbT_s[:, m2 * 128:(m2 + 1) * 128],
                                         rhs=lw.ones2_s[0:1, :],
                                         start=False, stop=True)
                    if dump and rr == 0:
                        for k in range(4):
                            nc.sync.dma_start(dbg_cs[:, k * 96:(k + 1) * 96], cs[k][:])
                            nc.sync.dma_start(dbg_mdd[:, k * 48:(k + 1) * 48], mdd_t[k][:])
                    s_new = st[rr + 4]
                    nc.vector.tensor_copy(out=s_new[:], in_=g_ps[:])
                    if rr < 7:
                        emit_uv(rr + 1, [3])
                    d_ps = sps.tile([4, 48], f32, tag="s")
                    for kc in range(2):
                        nc.tensor.matmul(d_ps[:], lhsT=lw.decT_s[:, kc * 4:kc * 4 + 4],
                                         rhs=s_new[:, kc * 48:kc * 48 + 48],
                                         start=(kc == 0), stop=False)
                    nc.tensor.matmul(d_ps[:], lhsT=lw.decbT_s[:],
                                     rhs=lw.ones2_s[0:1, :], start=False, stop=True)
                    bbv = bbox_sb[:].rearrange("f (b q) -> f b q", b=8)[:, :, rr * 6:rr * 6 + 6]
                    nc.vector.tensor_copy(out=bbv, in_=d_ps[:])
                    if rr < 7:
                        m = rr + 4
                        d2_ps = sps.tile([2, 48], f32, tag="s")
                        for kc in range(2):
                            nc.tensor.matmul(d2_ps[:], lhsT=lw.decT_s[:, kc * 4 + 2:kc * 4 + 4],
                                             rhs=s_new[:, kc * 48:kc * 48 + 48],
                                             start=(kc == 0), stop=False)
                        nc.tensor.matmul(d2_ps[:], lhsT=lw.decbT_s[:, 2:4],
                                         rhs=lw.ones2_s[0:1, :], start=False, stop=True)
                        coorb = hb.tile([2, 48], bf16, tag="coorb")
                        nc.vector.tensor_copy(out=coorb[:], in_=d2_ps[:])
                        cm2 = hb.tile([2, 48], bf16, tag="cm2")
                        nc.vector.tensor_scalar(out=cm2[:], in0=coorb[:], scalar1=-2.0,
                                                scalar2=None, op0=OP.mult)
                        sq = hb.tile([2, 48], bf16, tag="sq")
                        nc.vector.tensor_tensor(out=sq[:], in0=coorb[:], in1=coorb[:], op=OP.mult)
                        m_ps = sps.tile([112, 48], f32, tag="s")
                        nc.tensor.matmul(m_ps[64:112, :], lhsT=coorb[:], rhs=cm2[:],
                                         start=True, stop=False)
                        nc.tensor.matmul(m_ps[64:112, :], lhsT=sq[:], rhs=lw.ones2_s[:],
                                         start=False, stop=False)
                        nc.tensor.matmul(m_ps[64:112, :], lhsT=lw.ones2_s[:], rhs=sq[:],
                                         start=False, stop=True)
                        nc.vector.tensor_tensor(out=mdd_t[m][64:112, :], in0=m_ps[64:112, :],
                                                in1=lw.Tm_s[64:112, :], op=OP.is_le)
                        dd_ps = sps.tile([128, 48], f32, tag="s")
                        nc.tensor.matmul(dd_ps[:], lhsT=lw.ones48_s[64:112, :],
                                         rhs=mdd_t[m][64:112, :], start=True, stop=True)
                        nc.vector.tensor_tensor(out=mdd_t[m][0:48, :], in0=lw.eye48_s[:],
                                                in1=dd_ps[0:48, :], op=OP.mult)
                if dump:
                    for m in range(5):
                        nc.sync.dma_start(dbg_st[:, m * 96:(m + 1) * 96], st[m][:])
        for _rep in range(nrep):
            stages()
        if stage != 'full':
            nc.gpsimd.memset(bbox_sb[:], 0.0)
        nc.sync.dma_start(
            out[:].rearrange("b rr n f -> f (b rr n)"), bbox_sb[:])
    return nc


_NC = None

def _get_nc():
    global _NC
    if _NC is None:
        nc = bass.Bass()
        build(nc)
        split_drain_waits(nc)
        _NC = nc
    return _NC


def kernel(**inputs):
    nc = _get_nc()
    inputs = {k: np.asarray(v) for k, v in inputs.items()}
    maps = [make_core_inputs(inputs, s) for s in range(NCORE)]
    res = run_bass_kernel_spmd(nc, maps, core_ids=list(range(NCORE)))
    out = np.concatenate([res.results[s]["bbox_out"] for s in range(NCORE)], 0)
    return out.astype(np.float32)



# revision 62
# speedup vs baseline: 1.1152x; 1.1152x over previous
"""Trainium2 Bass kernel for nn_Net_63754494542044.

Data-parallel over 8 NeuronCores (8 B-samples each). Host pre-packs
conv1 im2col / conv weights / RoIAlign grid tables; device runs
conv1 -> conv2 -> RoIAlign gather+bilinear -> fc0/emb/red -> 8 GNN rollouts.
"""
import sys
sys.path.insert(0, '/opt/trn_rl_repo')
import numpy as np
from contextlib import ExitStack
import concourse.bass as bass
import concourse.tile as tile
from concourse import mybir
from concourse.bass_utils import run_bass_kernel_spmd

# Walrus wait-slot limits: CTRL-encoded (Drain/NoOp) = 1; others appear
# limited too on this build -- split conservatively.
def split_drain_waits(nc, max_waits=1, max_waits_other=1):
    for fn in nc.m.functions:
        for bb in fn.blocks:
            insts = bb.instructions
            i = 0
            while i < len(insts):
                inst = insts[i]
                si = getattr(inst, 'sync_info', None)
                lim = max_waits if isinstance(inst, (mybir.InstDrain, mybir.InstNoOp)) else max_waits_other
                if si is not None and si.on_wait and len(si.on_wait) > lim:
                    waits = list(si.on_wait)
                    keep = waits[-lim:]
                    extra = waits[:-lim]
                    new_nops = []
                    for k in range(0, len(extra), max_waits):
                        chunk = extra[k:k + max_waits]
                        nop = mybir.InstNoOp(
                            name=nc.get_next_instruction_name(),
                            engine=inst.engine,
                        )
                        nop.sync_info = mybir.SyncInfo(on_wait=chunk, on_update=[])
                        nc.register_instruction(nop)
                        new_nops.append(nop)
                    inst.sync_info = mybir.SyncInfo(on_wait=keep, on_update=list(si.on_update))
                    insts[i:i] = new_nops
                    i += len(new_nops)
                i += 1


import os
FP8_CONV = os.environ.get('BASSK_FP8', '1') == '1'

B, T, N = 64, 4, 6
IMG, CIN = 128, 3
VE, D, P = 64, 256, 4
SCALE = 0.25
NCORE = 8
BC = B // NCORE          # 8 samples per core
NIMG = BC * T            # 32 images per core
NROI = BC * T * N        # 192 rois per core
NROW = BC * N            # 48 gnn rows per core
NPT = NROI * 16          # 3072 sample points per core
NG = 24                  # gather groups


# ---------------- conv1 im2col (host) ----------------
# conv1: 3->64, 3x3, stride2, SAME on 128x128 -> 64x64.
# 2-px-packed output: out pair (oy, j) covers ox = 2j, 2j+1.
# K=45 rows: (rowtap rt in 0..2) x (coltap ct in 0..4) x (ci in 0..2)
#   input row for out oy: rt0: 2*oy-1, rt1: 2*oy, rt2: 2*oy+1
#   input col for out pair j: ct: 4j-1, 4j, 4j+1, 4j+2, 4j+3
def conv1_im2col_host(x):  # x [nimg, 3, 128, 128] fp32
    nimg = x.shape[0]
    xp = np.pad(x, ((0, 0), (0, 0), (0, 1), (0, 1)))  # SAME stride2: pad bottom/right only
    cols = np.empty((45, nimg, 64, 32), np.float32)
    k = 0
    for rt in range(3):
        for ct in range(5):
            for ci in range(3):
                # row = 2*oy + rt ; col = 4*j + ct
                cols[k] = xp[:, ci, rt:rt + 127:2, ct:ct + 125:4]
                k += 1
    return cols  # [45, nimg, 64, 32]


def conv1_weights_host(w_conv1):  # [64, 3, 3, 3]
    # W2 [45, 128]: col m = px*64 + oc ... out(oy, 2j+px) uses taps:
    #   orig tap (dy, dx): input row 2oy+dy-1 -> rt = dy ; input col 2(2j+px)+dx-1 = 4j + (2px+dx-1) -> ct = 2px+dx-1
    W2 = np.zeros((45, 128), np.float32)
    for px in range(2):
        for oc in range(64):
            m = px * 64 + oc
            for dy in range(3):
                for dx in range(3):
                    ct = 2 * px + dx
                    assert 0 <= ct <= 4  # input col = 4j + ct (no left pad)
                    for ci in range(3):
                        W2[(dy * 5 + ct) * 3 + ci, m] = w_conv1[oc, ci, dy, dx]
    return W2


def conv1_host(x, w_conv1, b_conv1):
    """Mirror of device conv1: returns feat1 [nimg, 64, 64, 64] (pre-relu + bias)."""
    cols = conv1_im2col_host(x)          # [45, nimg, 64, 32]
    W2 = conv1_weights_host(w_conv1)     # [45, 128]
    out = np.einsum('kf,kc->cf', cols.reshape(45, -1), W2)  # [128, nimg*64*32]
    out = out.reshape(2, 64, -1, 64, 32)  # [px, oc, img, oy, j]
    feat1 = np.empty((x.shape[0], 64, 64, 64), np.float32)
    feat1[..., 0::2] = np.transpose(out[0], (1, 0, 2, 3))
    feat1[..., 1::2] = np.transpose(out[1], (1, 0, 2, 3))
    feat1 += b_conv1[None, :, None, None]
    return feat1


# ---------------- conv2 weights (host) ----------------
# feat1_ph partitions: (px_in*64 + ci'), free (img, py, Y, X) halo X,Y in -1..31.
# conv2 out pair (oy2, j2): outs o1=2*j2, o2=2*j2+1 ; M col = pxo*64 + oc.
# 9 matmuls: rowtap r in {py0[Y], py1[Y-1], py1[Y]} x colgrp g in {X=j2 pair(K128), X=j2+?...}
# col groups: g0: pair (px0[Xa], px1[Xa]) Xa = j2? ... define by original dx:
#   out ox2: input x = 2*ox2 + dx - 1
#   for o1=2j2: x = 4j2-1, 4j2, 4j2+1 -> (px,X): (1, 2j2-1), (0, 2j2), (1, 2j2)
#   for o2=2j2+1: x = 4j2+1, 4j2+2, 4j2+3 -> (1, 2j2), (0, 2j2+1), (1, 2j2+1)
# X taps: px0: {2j2, 2j2+1} ; px1: {2j2-1, 2j2, 2j2+1}
# col groups (relative X offset from base 2j2):
#   gA: K128 = (px0[2j2], px1[2j2])        -> X offset 0, both phases
#   gB: K128 = (px0[2j2+1], px1[2j2+1])    -> X offset +1, both phases
#   gC: K64  = px1[2j2-1]                  -> X offset -1, px1 only
# rowtaps r (input y = 2*oy2 + dy - 1):
#   dy0: y = 2oy2-1 -> (py1, Y=oy2-1) ; dy1: y=2oy2 -> (py0, Y=oy2) ; dy2: y=2oy2+1 -> (py1, Y=oy2)
def conv2_weights_host(w_conv2):  # [64, 64, 3, 3]
    # Wb[r][g]: gA/gB: [128, 128] (partition = pxi*64+ci), gC: [64, 128]
    # dy maps to rowtap r directly (r=0: dy=0 ; r=1: dy=1 ; r=2: dy=2)
    Wb = [[np.zeros((128, 128), np.float32) for _ in range(2)] + [np.zeros((64, 128), np.float32)]
          for _ in range(3)]
    for pxo in range(2):          # which output in the pair (o = 2j2+pxo)
        for oc in range(64):
            m = pxo * 64 + oc
            for dy in range(3):
                for dx in range(3):
                    x_off = 2 * pxo + dx     # input x = 4j2 + x_off, x_off in 0..4
                    pxi = x_off % 2
                    Xrel = x_off // 2        # in {0, 1, 2}
                    for ci in range(64):
                        if Xrel < 2:
                            Wb[dy][Xrel][pxi * 64 + ci, m] += w_conv2[oc, ci, dy, dx]
                        else:
                            assert pxi == 0
                            Wb[dy][2][ci, m] += w_conv2[oc, ci, dy, dx]
    return Wb


def conv2_host(feat1r, w_conv2, b_conv2):
    """feat1r: relu'd feat1 [nimg, 64, 64, 64]. Returns feat2 [nimg, 64, 32, 32] pre-relu."""
    nimg = feat1r.shape[0]
    # build feat1_ph with halo: [128 part (pxi*64+ci), img, py, Y(-1..31), X(-1..31)]
    ph = np.zeros((128, nimg, 2, 33, 33), np.float32)  # halo at Y=32, X=32
    f = feat1r  # [img, ci, y, x]
    for pxi in range(2):
        for py in range(2):
            ph[pxi * 64:pxi * 64 + 64, :, py, :32, :32] = np.transpose(
                f[:, :, py::2, pxi::2], (1, 0, 2, 3))
    Wb = conv2_weights_host(w_conv2)
    out = np.zeros((128, nimg, 32, 16), np.float32)  # [(pxo,oc), img, oy2, j2]
    # rowtap dy: input y = 2*oy2 + dy -> (py = dy&1, Y = oy2 + dy//2)
    for dy in range(3):
        py, Yoff = dy % 2, dy // 2
        for g in range(3):
            W = Wb[dy][g]
            Ysl = slice(Yoff, Yoff + 32)
            Xidx = g + 2 * np.arange(16)   # X = 2*j2 + Xrel ... stored X index = that
            rhs = ph[:, :, py, Ysl, :][:, :, :, Xidx]  # [128 or .., img, 32, 16]
            if g == 2:
                rhs = rhs[:64]
            out += np.einsum('km,kijx->mijx', W, rhs)
    feat2 = np.empty((nimg, 64, 32, 32), np.float32)
    feat2[..., 0::2] = np.transpose(out[:64], (1, 0, 2, 3))
    feat2[..., 1::2] = np.transpose(out[64:], (1, 0, 2, 3))
    return feat2 + b_conv2[None, :, None, None]


# ---------------- RoIAlign grid (host) ----------------
def roi_grid_host(rois):  # rois [NROI, 5] fp32 (batch-local; bidx = local img idx)
    """Returns idx int32 [NPT, 2] (row-gather indices, row=(img,y,j2) width 128),
    weights w4 [NPT, 4] fp32 (w00,w01,w10,w11 order: (y0x0, y0x1, y1x0, y1x1)),
    parity [NPT] (x0&1)."""
    nroi = rois.shape[0]
    W = H = 32
    x1 = rois[:, 1] * SCALE; y1 = rois[:, 2] * SCALE
    x2 = rois[:, 3] * SCALE; y2 = rois[:, 4] * SCALE
    bw = np.maximum(x2 - x1, 1.0) / P
    bh = np.maximum(y2 - y1, 1.0) / P
    grid = np.arange(P, dtype=np.float32) + 0.5
    sx = x1[:, None, None] + bw[:, None, None] * grid[None, None, :]   # [R, P(py), P(px)]
    sy = y1[:, None, None] + bh[:, None, None] * grid[None, :, None]
    sx = np.broadcast_to(sx, (nroi, P, P)).reshape(-1)
    sy = np.broadcast_to(sy, (nroi, P, P)).reshape(-1)
    x0f = np.clip(np.floor(sx), 0, W - 1)
    y0f = np.clip(np.floor(sy), 0, H - 1)
    lx = np.clip(sx - x0f, 0.0, 1.0)
    ly = np.clip(sy - y0f, 0.0, 1.0)
    # clamp x0 to <= 30 adjusting lx (exact when sx>=31: both corners read col 31)
    x0 = x0f.astype(np.int32); y0 = y0f.astype(np.int32)
    hi = x0 >= 31
    x0 = np.where(hi, 30, x0); lx = np.where(hi, 1.0, lx).astype(np.float32)
    hiy = y0 >= 31
    y0 = np.where(hiy, 30, y0); ly = np.where(hiy, 1.0, ly).astype(np.float32)
    img = np.repeat(np.arange(nroi, dtype=np.int32) // N, 16)
    j2 = x0 >> 1
    par = (x0 & 1).astype(np.float32)
    idx0 = img * 512 + y0 * 16 + j2          # row idx (rows of 128 els)
    idx1 = idx0 + 16                          # y0+1 row
    w4 = np.stack([(1 - ly) * (1 - lx), (1 - ly) * lx, ly * (1 - lx), ly * lx], 1).astype(np.float32)
    return np.stack([idx0, idx1], 1).astype(np.int32), w4, par


def roi_wmat_host(rois):
    """RoIAlign as per-image matmul: sparse bilinear weights densified.
    Returns Wg [NIMG, 8, 128, 96] f32: for image i, chunk c = b*2+px
    (b = pair block 0..3, px = x parity), Wg[i, c, pair_local, n*16+pt] =
    bilinear weight of pixel (y, x) for point pt of roi n, where
    pair = y*16 + (x>>1) = b*128 + pair_local."""
    nroi = rois.shape[0]
    x1 = rois[:, 1] * SCALE; y1 = rois[:, 2] * SCALE
    x2 = rois[:, 3] * SCALE; y2 = rois[:, 4] * SCALE
    bw = np.maximum(x2 - x1, 1.0) / P
    bh = np.maximum(y2 - y1, 1.0) / P
    grid = np.arange(P, dtype=np.float32) + 0.5
    sx = x1[:, None, None] + bw[:, None, None] * grid[None, None, :]
    sy = y1[:, None, None] + bh[:, None, None] * grid[None, :, None]
    sx = np.broadcast_to(sx, (nroi, P, P)).reshape(-1)
    sy = np.broadcast_to(sy, (nroi, P, P)).reshape(-1)
    x0f = np.clip(np.floor(sx), 0, 31); y0f = np.clip(np.floor(sy), 0, 31)
    lx = np.clip(sx - x0f, 0.0, 1.0).astype(np.float32)
    ly = np.clip(sy - y0f, 0.0, 1.0).astype(np.float32)
    x0 = x0f.astype(np.int64); y0 = y0f.astype(np.int64)
    x1i = np.minimum(x0 + 1, 31); y1i = np.minimum(y0 + 1, 31)
    img = np.arange(nroi).repeat(16) // N
    col = (np.arange(nroi) % N).repeat(16) * 16 + np.tile(np.arange(16), nroi)
    Wg = np.zeros((NIMG, 8, 128, 96), np.float32)
    flat = Wg.reshape(-1)
    for w, yy, xx in (((1 - ly) * (1 - lx), y0, x0), ((1 - ly) * lx, y0, x1i),
                      (ly * (1 - lx), y1i, x0), (ly * lx, y1i, x1i)):
        pair = yy * 16 + (xx >> 1)
        c = (pair >> 7) * 2 + (xx & 1)
        idxf = ((img * 8 + c) * 128 + (pair & 127)) * 96 + col
        np.add.at(flat, idxf, w)
    return Wg


def roi_align_host(feat2r, rois):
    """Mirror of device pool-matmul -> pooled [NPT, 64] pt-major."""
    Wg = roi_wmat_host(rois)                       # [NIMG, 8, 128, 96]
    # F2c[i, c, pl, ch] = feat2r[i, ch, y, x], c = (y//8)*2 + (x&1),
    # pl = (y%8)*16 + (x>>1)
    f = feat2r.reshape(NIMG, 64, 4, 8, 16, 2)       # [i, ch, b, y8, j2, px]
    F2c = np.transpose(f, (0, 2, 5, 3, 4, 1)).reshape(NIMG, 4, 2, 128, 64)
    F2c = F2c.reshape(NIMG, 8, 128, 64)             # chunk order (b, px) ✓
    pooled = np.einsum('icpn,icpm->inm', Wg, F2c)   # [i, 96, 64]
    return pooled.reshape(NPT, 64)


# ---------------- GNN (host mirror of device algebra) ----------------
def mask_host(coor, r):
    """coor [BC, N, 2], r [BC, N] -> bigmask [NROW, NROW] fp32 block-diag, deg [NROW]."""
    bm = np.zeros((NROW, NROW), np.float32)
    for b in range(BC):
        d = np.linalg.norm(coor[b][:, None, :] - coor[b][None, :, :], axis=-1)
        m = (d <= (r[b][:, None] + r[b][None, :])) & ~np.eye(N, dtype=bool)
        bm[b * N:(b + 1) * N, b * N:(b + 1) * N] = m
    return bm, bm.sum(1)


def internet_host(s, bm, deg, p):
    """s [NROW, D] fp32 row-major; bm [NROW,NROW]; p = (sw,sb,rw,rb,aw,ab,ow,ob)."""
    sw, sb, rw, rb, aw, ab, ow, ob = p
    Wl, Wr = rw[:, :D], rw[:, D:]
    self_d = s @ sw.T + sb
    u = s @ Wl.T + rb
    v = s @ Wr.T
    rel = deg[:, None] * u + bm @ v
    a = np.maximum((self_d + rel) @ aw.T + ab, 0)
    return np.maximum(a @ ow[:, :D].T + s @ ow[:, D:].T + ob, 0)


def gnn_host(obj_t, src_coor, r, inputs):
    """obj_t [4][NROW, D] initial states; src_coor [BC, T, N, 2]; r [BC, N].
    Returns bboxes [BC, 8, N, 4]."""
    states = list(obj_t)
    masks = [mask_host(src_coor[:, t], r) for t in range(4)]
    num_rollouts = int(inputs['num_rollouts'])
    out = []
    for rr in range(num_rollouts):
        cs = []
        for k in range(4):
            p = (inputs['g_self_w'][k], inputs['g_self_b'][k], inputs['g_rel_w'][k],
                 inputs['g_rel_b'][k], inputs['g_aff_w'][k], inputs['g_aff_b'][k],
                 inputs['g_out_w'][k], inputs['g_out_b'][k])
            bm, deg = masks[k]
            cs.append(internet_host(states[k], bm, deg, p))
        s = np.concatenate(cs, -1) @ inputs['agg_w'].T + inputs['agg_b']
        bbox = s @ inputs['dec_w'].T + inputs['dec_b']          # [NROW, 4]
        out.append(bbox.reshape(BC, N, 4))
        states = states[1:] + [s]
        coor = bbox[:, 2:].reshape(BC, N, 2)
        masks = masks[1:] + [mask_host(coor, r)]
    return np.stack(out, 1)


def full_host(inputs, shard):
    """Complete per-core mirror (fp32). shard = B-slice index."""
    sl = slice(shard * BC, (shard + 1) * BC)
    x = inputs['x'][sl].reshape(NIMG, CIN, IMG, IMG)
    rois = inputs['rois'][sl].reshape(NROI, 5)
    coor = inputs['src_coor_features'][sl]                      # [BC, T, N, 2]
    r = (((rois.reshape(BC, T, N, 5)[..., 4] - rois.reshape(BC, T, N, 5)[..., 2]) / 2
          + (rois.reshape(BC, T, N, 5)[..., 3] - rois.reshape(BC, T, N, 5)[..., 1]) / 2) / 2).mean(1)
    f1 = np.maximum(conv1_host(x, inputs['w_conv1'], inputs['b_conv1']), 0)
    f2 = np.maximum(conv2_host(f1, inputs['w_conv2'], inputs['b_conv2']), 0)
    pooled = roi_align_host(f2, rois)                           # [NPT, 64] pt-major
    # fc0: obj[row, d] = sum_{c,pt} pool[row, pt, c] * fc0_w[d, c*16+pt]
    pool_cp = pooled.reshape(NROI, 16, 64)
    Wp = inputs['fc0_w'].reshape(D, 64, 16)                     # [d, c, pt]
    obj = np.einsum('rpc,dcp->rd', pool_cp, Wp) + inputs['fc0_b']
    obj = np.maximum(obj, 0)                                    # [NROI, D] rows (b,t,n)
    emb = np.maximum(coor.reshape(NROI, 2) @ inputs['fc0c_w'].T + inputs['fc0c_b'], 0)
    emb = np.maximum(emb @ inputs['fc1c_w'].T + inputs['fc1c_b'], 0)
    o2 = np.maximum(obj @ inputs['red_w'][:, :D].T + emb @ inputs['red_w'][:, D:].T
                    + inputs['red_b'], 0)                       # [NROI, D]
    o2 = o2.reshape(BC, T, N, D)
    obj_t = [o2[:, t].reshape(NROW, D) for t in range(4)]
    return gnn_host(obj_t, coor, r, inputs)


# ---------------- device input packing ----------------
def make_core_inputs(inputs, shard):
    import ml_dtypes
    bf16 = ml_dtypes.bfloat16
    sl = slice(shard * BC, (shard + 1) * BC)
    x = np.asarray(inputs['x'][sl], np.float32).reshape(NIMG, CIN, IMG, IMG)
    rois = np.asarray(inputs['rois'][sl], np.float32).reshape(NROI, 5)
    coor = np.asarray(inputs['src_coor_features'][sl], np.float32)   # [BC,T,N,2]
    rr5 = rois.reshape(BC, T, N, 5)
    r = (((rr5[..., 4] - rr5[..., 2]) / 2 + (rr5[..., 3] - rr5[..., 1]) / 2) / 2).mean(1)

    fp8 = ml_dtypes.float8_e4m3
    cdt = fp8 if FP8_CONV else bf16
    d = {}
    cols = conv1_im2col_host(x).reshape(45, -1)       # [45, NIMG*64*32]
    if FP8_CONV:
        c46 = np.zeros((46, cols.shape[1]), np.float32)
        c46[:45] = cols
        # DoubleRow pair layout: row p holds tap p | tap 23+p side by side
        d['im2col45'] = np.concatenate([c46[:23], c46[23:]], 1).astype(fp8)
        w46 = np.zeros((46, 128), np.float32)
        w46[:45] = conv1_weights_host(np.asarray(inputs['w_conv1']))
        d['w1'] = np.concatenate([w46[:23], w46[23:]], 1).astype(fp8)
    else:
        d['im2col45'] = cols.astype(bf16)
        d['w1'] = conv1_weights_host(np.asarray(inputs['w_conv1'])).astype(bf16)
    b1 = np.asarray(inputs['b_conv1'], np.float32)
    d['b1'] = np.tile(b1, 2).reshape(128, 1).astype(np.float32)
    Wb = conv2_weights_host(np.asarray(inputs['w_conv2']))
    if FP8_CONV:
        # DoubleRow pairs: P0=[gA-dy0,gA-dy1], P1=[gB-dy0,gB-dy1],
        # P2=[gA-dy2,gB-dy2]; gC stays as 3 singles whose x-wrap at j2=15
        # is cancelled by small correction matmuls with negated weights
        d['w2p'] = np.stack([
            np.stack([Wb[0][0], Wb[1][0]], 1).reshape(128, 256),
            np.stack([Wb[0][1], Wb[1][1]], 1).reshape(128, 256),
            np.stack([Wb[2][0], Wb[2][1]], 1).reshape(128, 256)]).astype(cdt)
    else:
        d['w2a'] = np.stack([Wb[dy][0] for dy in range(3)]).astype(cdt)
        d['w2b'] = np.stack([Wb[dy][1] for dy in range(3)]).astype(cdt)
    d['w2c'] = np.stack([Wb[dy][2] for dy in range(3)]).astype(cdt)
    d['w2cn'] = np.stack([-Wb[dy][2] for dy in range(3)]).astype(cdt)
    b2 = np.asarray(inputs['b_conv2'], np.float32)
    d['b2'] = np.tile(b2, 2).reshape(128, 1).astype(np.float32)

    Wg = roi_wmat_host(rois)                          # [NIMG, 8, 128, 96]
    # device layout per group g: [128 pair_local, (img_local, chunk, pt) 6144]
    d['wroi'] = np.ascontiguousarray(
        Wg.reshape(NGRP, IMG_GRP, 8, 128, 96).transpose(0, 3, 1, 2, 4)
        .reshape(NGRP, 128, IMG_GRP * 8 * 96)).astype(bf16)

    fc0w = np.asarray(inputs['fc0_w'], np.float32).reshape(D, 64, 16)  # [d, c, pt]
    d['fc0t'] = np.ascontiguousarray(fc0w.transpose(2, 1, 0)).astype(bf16)  # [pt, c, d]
    d['fc0b'] = np.asarray(inputs['fc0_b'], np.float32).reshape(2, 128).T.copy()

    d['coor_fm'] = coor.reshape(NROI, 2).T.astype(bf16).copy()

    def t2(w):   # [256, K] -> [kc, 128, 256] lhsT chunks (w.T row-chunks)
        wT = np.ascontiguousarray(np.asarray(w, np.float32).T)       # [K, 256]
        K = wT.shape[0]
        return wT.reshape(K // 128, 128, 256).astype(bf16)

    def bcol(b):  # [256] -> [128, 2]
        return np.asarray(b, np.float32).reshape(2, 128).T.copy()

    d['fc0ct'] = np.asarray(inputs['fc0c_w'], np.float32).T.astype(bf16).copy()  # [2, 256]
    d['fc0cb'] = bcol(inputs['fc0c_b'])
    d['fc1ct'] = t2(inputs['fc1c_w'])
    d['fc1cb'] = bcol(inputs['fc1c_b'])
    redw = np.asarray(inputs['red_w'], np.float32)
    d['redoT'] = t2(redw[:, :D])
    d['redeT'] = t2(redw[:, D:])
    d['redb'] = bcol(inputs['red_b'])

    d['gswT'] = np.stack([t2(inputs['g_self_w'][k]) for k in range(4)])
    grw = np.asarray(inputs['g_rel_w'], np.float32)
    d['gWlT'] = np.stack([t2(grw[k][:, :D]) for k in range(4)])
    d['gWrT'] = np.stack([t2(grw[k][:, D:]) for k in range(4)])
    d['gawT'] = np.stack([t2(inputs['g_aff_w'][k]) for k in range(4)])
    gow = np.asarray(inputs['g_out_w'], np.float32)
    d['gowaT'] = np.stack([t2(gow[k][:, :D]) for k in range(4)])
    d['gowsT'] = np.stack([t2(gow[k][:, D:]) for k in range(4)])

    d['gbiasT'] = np.concatenate([
        np.asarray(inputs['g_self_b'], np.float32).reshape(-1),
        np.asarray(inputs['g_aff_b'], np.float32).reshape(-1),
        np.asarray(inputs['g_out_b'], np.float32).reshape(-1)]).reshape(1, 3072).astype(bf16)
    d['rbT'] = np.asarray(inputs['g_rel_b'], np.float32).reshape(1, 1024).astype(bf16)
    d['zrow'] = np.zeros((1, 256), bf16)
    d['aggT'] = t2(inputs['agg_w'])                    # [8, 128, 256]
    d['aggbT'] = np.asarray(inputs['agg_b'], np.float32).reshape(1, 256).astype(bf16)
    decw = np.asarray(inputs['dec_w'], np.float32)     # [4, 256]
    d['decT'] = decw.T.reshape(2, 128, 4).astype(bf16).copy()
    decb = np.asarray(inputs['dec_b'], np.float32).reshape(4)
    # col 0 = dec_b (for d_ps, partitions 0-3); col 1 rows 0-1 = dec_b[2:4]
    # (for d2_ps which lives on partitions 0-1)
    d['decb4'] = np.stack([decb, np.concatenate([decb[2:4], [0., 0.]])], 1)

    hmdds = []
    for m in range(4):
        bm, deg = mask_host(coor[:, m], r)
        mdd = np.zeros((112, NROW), np.float32)
        mdd[0:48] = np.diag(deg)
        mdd[64:112] = bm
        hmdds.append(mdd.astype(bf16))
    d['hmdd'] = np.stack(hmdds)
    Tmat = np.full((NROW, NROW), -1.0, np.float32)
    for b in range(BC):
        rs = (r[b][:, None] + r[b][None, :]) ** 2
        np.fill_diagonal(rs, -1.0)
        Tmat[b * N:(b + 1) * N, b * N:(b + 1) * N] = rs
    Tm112 = np.zeros((112, NROW), np.float32)
    Tm112[64:112] = Tmat
    d['Tm'] = Tm112
    d['ones48'] = np.ones((112, 128), bf16)
    d['ones2'] = np.ones((2, 48), bf16)
    d['ident'] = np.eye(128, dtype=bf16)
    d['eye48'] = np.eye(48, dtype=np.float32)
    return d


dt = mybir.dt
AF = mybir.ActivationFunctionType
OP = mybir.AluOpType

NIMG, NROI, NROW, NPT = 32, 192, 48, 3072
NG = 24            # gather groups (128 pts each)
IMG_GRP = 8        # images per conv group
NGRP = NIMG // IMG_GRP
IMGF = 2 * 33 * 32  # 2112 free els per img in feat1 (x tight, y has a zero row)


def build(nc: bass.Bass, dump=False, stage='full', nrep=1):
    f32, bf16, i32 = dt.float32, dt.bfloat16, dt.int32
    f8 = dt.float8e4 if FP8_CONV else dt.bfloat16

    def din(name, shape, d):
        return nc.dram_tensor(name, shape, d, kind="ExternalInput")

    if FP8_CONV:
        im2col = din("im2col45", [23, 131072], f8)
        w1 = din("w1", [23, 256], f8)
    else:
        im2col = din("im2col45", [45, 65536], f8)
        w1 = din("w1", [45, 128], f8)
    b1 = din("b1", [128, 1], f32)
    if FP8_CONV:
        w2p = din("w2p", [3, 128, 256], f8)
    else:
        w2a = din("w2a", [3, 128, 128], f8)
        w2b = din("w2b", [3, 128, 128], f8)
    w2c = din("w2c", [3, 64, 128], f8)
    w2cn = din("w2cn", [3, 64, 128], f8)
    b2 = din("b2", [128, 1], f32)
    wroi = din("wroi", [NGRP, 128, IMG_GRP * 8 * 96], bf16)
    fc0t = din("fc0t", [16, 64, 256], bf16)
    fc0b = din("fc0b", [128, 2], f32)
    coor = din("coor_fm", [2, 192], bf16)
    fc0ct = din("fc0ct", [2, 256], bf16)
    fc0cb = din("fc0cb", [128, 2], f32)
    fc1ct = din("fc1ct", [2, 128, 256], bf16)
    fc1cb = din("fc1cb", [128, 2], f32)
    redoT = din("redoT", [2, 128, 256], bf16)
    redeT = din("redeT", [2, 128, 256], bf16)
    redb = din("redb", [128, 2], f32)
    gswT = din("gswT", [4, 2, 128, 256], bf16)
    gWlT = din("gWlT", [4, 2, 128, 256], bf16)
    gWrT = din("gWrT", [4, 2, 128, 256], bf16)
    gawT = din("gawT", [4, 2, 128, 256], bf16)
    gowaT = din("gowaT", [4, 2, 128, 256], bf16)
    gowsT = din("gowsT", [4, 2, 128, 256], bf16)
    gbiasT = din("gbiasT", [1, 3072], bf16)
    rbT = din("rbT", [1, 1024], bf16)
    zrow = din("zrow", [1, 256], bf16)
    aggT = din("aggT", [8, 128, 256], bf16)
    aggbT = din("aggbT", [1, 256], bf16)
    decT = din("decT", [2, 128, 4], bf16)
    decb4 = din("decb4", [4, 2], f32)
    hmdd = din("hmdd", [4, 112, 48], bf16)
    Tm = din("Tm", [112, 48], f32)
    ones48 = din("ones48", [112, 128], bf16)
    ones2 = din("ones2", [2, 48], bf16)
    ident = din("ident", [128, 128], bf16)
    eye48 = din("eye48", [48, 48], f32)

    out = nc.dram_tensor("bbox_out", [8, 8, 6, 4], f32, kind="ExternalOutput")
    if dump:
        dbg_mdd = nc.dram_tensor("dbg_mdd", [112, 192], bf16, kind="ExternalOutput")
        dbg_uvt = nc.dram_tensor("dbg_uvt", [112, 1024], bf16, kind="ExternalOutput")
        dbg_cs = nc.dram_tensor("dbg_cs", [128, 384], bf16, kind="ExternalOutput")
        dbg_st = nc.dram_tensor("dbg_st", [128, 480], bf16, kind="ExternalOutput")
        dbg_x = nc.dram_tensor("dbg_x", [128, 384], bf16, kind="ExternalOutput")
        dbg_a = nc.dram_tensor("dbg_a", [128, 384], bf16, kind="ExternalOutput")

    with tile.TileContext(nc) as tc, ExitStack() as ctx:
        # ---- persistent pools ----
        wp = ctx.enter_context(tc.tile_pool(name="w", bufs=1))
        sp = ctx.enter_context(tc.tile_pool(name="state", bufs=1))

        def load(dram_t, shape, dtype, src_ap=None):
            t = wp.tile(shape, dtype, tag=dram_t.name)
            if src_ap is None:
                nc.sync.dma_start(t[:], dram_t[:, :])
            else:
                # src_ap dims [p, d0, d1, ...]; dst = t reshaped to match
                dims = [c for _, c in src_ap.ap[1:]]
                spec = " ".join(f"d{i}" for i in range(len(dims)))
                kw = {f"d{i}": dims[i] for i in range(len(dims) - 1)}
                dv = t[:].rearrange(f"p ({spec}) -> p {spec}", **kw)
                nc.sync.dma_start(dv, src_ap)
            return t

        # conv-critical loads first so im2col g0 isn't queued behind ~4MB of
        # GNN weights; everything else loads mid-body, overlapped with conv.
        w1_s = load(w1, [23, 256] if FP8_CONV else [45, 128], f8)
        b1_s = load(b1, [128, 1], f32)
        if FP8_CONV:
            w2p_s = load(w2p, [128, 768], f8, w2p[:].rearrange("q p m -> p q m"))
        else:
            w2a_s = load(w2a, [128, 3 * 128], f8, w2a[:].rearrange("d p m -> p d m"))
            w2b_s = load(w2b, [128, 3 * 128], f8, w2b[:].rearrange("d p m -> p d m"))
        w2c_s = load(w2c, [64, 3 * 128], f8, w2c[:].rearrange("d p m -> p d m"))
        w2cn_s = load(w2cn, [64, 3 * 128], f8, w2cn[:].rearrange("d p m -> p d m"))
        b2_s = load(b2, [128, 1], f32)
        ident_s = load(ident, [128, 128], bf16)

        class _LW: pass
        lw = _LW()

        def loadg(t):  # [4,2,128,256] -> [128, 4*512]
            return load(t, [128, 2048], bf16, t[:].rearrange("h k p m -> p h k m"))

        # GNN/fc weight loads split into per-conv-group chunks so the DMA
        # overlaps the conv stage instead of stalling fc0/rollout-0, while
        # never queueing ahead of that group's conv-critical im2col/wroi
        def late_c0():
            lw.fc0t_s = load(fc0t, [64, 16 * 256], bf16,
                                  fc0t[:].rearrange("t p m -> p t m"))
            lw.fc0b_s = load(fc0b, [128, 2], f32)
            lw.coor_s = load(coor, [2, 192], bf16)
            lw.fc0ct_s = load(fc0ct, [2, 256], bf16)
            lw.fc0cb_s = load(fc0cb, [128, 2], f32)
            lw.fc1ct_s = load(fc1ct, [128, 512], bf16,
                                   fc1ct[:].rearrange("k p m -> p k m"))
            lw.fc1cb_s = load(fc1cb, [128, 2], f32)
            lw.redoT_s = load(redoT, [128, 512], bf16,
                                   redoT[:].rearrange("k p m -> p k m"))
            lw.redeT_s = load(redeT, [128, 512], bf16,
                                   redeT[:].rearrange("k p m -> p k m"))
            lw.redb_s = load(redb, [128, 2], f32)
            for m in range(4):
                nc.sync.dma_start(mdd_t[m][:], hmdd[m])
            for m in range(4, 11):
                nc.gpsimd.memset(mdd_t[m][32:64, :], 0.0)

        def late_c1():
            lw.gswT_s, lw.gWlT_s, lw.gWrT_s = loadg(gswT), loadg(gWlT), loadg(gWrT)
            lw.gbiasT_s = load(gbiasT, [1, 3072], bf16)
            lw.rbT_s = load(rbT, [1, 1024], bf16)
            lw.zrow_s = load(zrow, [1, 256], bf16)
            lw.Tm_s = load(Tm, [112, 48], f32)
            lw.ones48_s = load(ones48, [112, 128], bf16)
            lw.ones2_s = load(ones2, [2, 48], bf16)
            lw.eye48_s = load(eye48, [48, 48], f32)

        def late_c2():
            lw.gawT_s, lw.gowaT_s = loadg(gawT), loadg(gowaT)

        def late_c3():
            lw.gowsT_s = loadg(gowsT)
            lw.aggT_s = load(aggT, [128, 2048], bf16,
                                  aggT[:].rearrange("k p m -> p k m"))
            lw.aggbT_s = load(aggbT, [1, 256], bf16)
            lw.decT_s = load(decT, [128, 8], bf16,
                                  decT[:].rearrange("k p m -> p k m"))
            lw.decb4_s = load(decb4, [4, 2], f32)

        late_chunks = [late_c0, late_c1, late_c2, late_c3]


        # mask/ddiag slots [112,48]: rows 0-47 diag(deg), 48-63 zero,
        # 64-111 mask (engine writes must start at partition 0/32/64/96)
        mdd_t = [sp.tile([112, 48], bf16, name=f"mdd{m}", tag=f"mdd{m}") for m in range(11)]
        # per-head [u+rb; 0; v] lhsT tiles (rel bias folded in via K=1 matmul)
        uvt = [sp.tile([112, 256], bf16, name=f"uvt{k}", tag=f"uvt{k}") for k in range(4)]

        st = [sp.tile([128, 96], bf16, name=f"st{m}", tag=f"st{m}") for m in range(12)]
        bbox_sb = sp.tile([4, 384], f32, tag="bbox")
        poolT = sp.tile([64, 3072], bf16, tag="poolT")

        def stages():
            if stage == 'setup':
                return

            # ================= conv stage =================
            with ExitStack() as cvx:
                imcp = cvx.enter_context(tc.tile_pool(name="imc", bufs=2))
                f1p = cvx.enter_context(tc.tile_pool(name="f1", bufs=2))
                c1ps = cvx.enter_context(tc.tile_pool(name="c1ps", bufs=3, space="PSUM"))
                c2ps = cvx.enter_context(tc.tile_pool(name="c2ps", bufs=2, space="PSUM"))
                tps = cvx.enter_context(tc.tile_pool(name="tps", bufs=1, space="PSUM"))
                pps = cvx.enter_context(tc.tile_pool(name="pps", bufs=1, space="PSUM"))
                cfps = cvx.enter_context(tc.tile_pool(name="cfps", bufs=1, space="PSUM"))
                f2p = cvx.enter_context(tc.tile_pool(name="f2", bufs=3))
                wrp = cvx.enter_context(tc.tile_pool(name="wr", bufs=2))

                GC = IMG_GRP * 2048
                for g in range(NGRP):
                    if FP8_CONV:
                        imc = imcp.tile([23, 2 * IMG_GRP * 2048], f8, tag="imc")
                        imv = imc[:].rearrange("p (i n) -> p i n", i=2)
                        nc.sync.dma_start(imv[:, 0, :], im2col[:, g * GC:(g + 1) * GC])
                        nc.sync.dma_start(imv[:, 1, :],
                                          im2col[:, 65536 + g * GC:65536 + (g + 1) * GC])
                    else:
                        imc = imcp.tile([45, IMG_GRP * 2048], f8, tag="imc")
                        nc.sync.dma_start(imc[:], im2col[:, g * GC:(g + 1) * GC])
                    f1 = f1p.tile([128, IMG_GRP * IMGF], f8, tag="f1")
                    # layout (py, y33, x32): x tight so the conv2 column walk
                    # collapses to one AP dim; y=32 is a zero row for the dy2
                    # taps (gC's x-wrap at j2=15 is cancelled by corrections)
                    f1h = f1[:].rearrange("p (i py y x) -> p i py y x", i=IMG_GRP, py=2, y=33, x=32)
                    nc.gpsimd.memset(f1h[:, :, :, 32:33, :], 0.0)
                    for i in range(IMG_GRP):
                        # conv1: 4 matmuls of [45,128]x[45,512], each its own
                        # 1-bank psum group; evac relu+bias per group covers
                        # both py phases (psum cols (py2, y8, j32) -> f1)
                        for g4 in range(4):
                            ps = c1ps.tile([128, 512], f32, tag="c1")
                            off = i * 2048 + g4 * 512
                            if FP8_CONV:
                                rhs = bass.AP(imc[:].tensor, off,
                                              [imc[:].ap[0], [16384, 2], [1, 512]])
                                nc.tensor.matmul(ps[:],
                                                 lhsT=w1_s[:].rearrange(
                                                     "p (i m) -> p i m", i=2),
                                                 rhs=rhs, start=True, stop=True,
                                                 perf_mode=mybir.MatmulPerfMode.DoubleRow)
                            else:
                                nc.tensor.matmul(ps[:],
                                                 lhsT=w1_s[:],
                                                 rhs=imc[:, off:off + 512],
                                                 start=True, stop=True)
                            ps_t = ps[:]
                            src = bass.AP(ps_t.tensor, ps_t.offset,
                                          [ps_t.ap[0], [32, 2], [64, 8], [1, 32]])
                            dst = f1h[:, i, :, 8 * g4:8 * g4 + 8, 0:32]
                            if g4 % 2 == 0:
                                nc.vector.tensor_scalar(
                                    out=dst, in0=src, scalar1=b1_s[:, 0:1],
                                    scalar2=0.0, op0=OP.add, op1=OP.max)
                            else:
                                nc.scalar.activation(out=dst, in_=src,
                                                     func=AF.Relu, bias=b1_s[:, 0:1])
                    for i in range(IMG_GRP):
                        # conv2: 9 matmuls -> psum [128, 512] cols (oy2 32, j2 16)
                        wr = wrp.tile([128, 8 * 96], bf16, tag="wr")
                        nc.sync.dma_start(wr[:], wroi[g][:, i * 768:(i + 1) * 768])
                        ps = c2ps.tile([128, 512], f32, tag="c2")
                        fb = f1[:]          # free layout (i, py 2, y 33, x 32)
                        ib = i * IMGF       # image base offset
                        PYS = 33 * 32       # py plane stride

                        def c2walk(py, yo, x0, pair=None):
                            # collapsed column walk: psum col (oy2, j2) reads
                            # f1[py, yo+oy2, x0+2*j2] = base + 2*(oy2*16+j2)
                            off = ib + py * PYS + yo * 32 + x0
                            dims = [fb.ap[0]]
                            if pair is not None:
                                dims.append([pair, 2])
                            dims.append([2, 512])
                            return bass.AP(fb.tensor, fb.offset + off, dims)

                        if FP8_CONV:
                            # fp8 DoubleRow: gA/gB dy0+dy1 paired across py
                            # planes (stride PYS), gA-dy2+gB-dy2 paired
                            # across x (stride 1)
                            DR = mybir.MatmulPerfMode.DoubleRow

                            def w2pair(q):
                                return w2p_s[:, q * 256:(q + 1) * 256].rearrange(
                                    "p (i m) -> p i m", i=2)
                            nc.tensor.matmul(ps[:], lhsT=w2pair(0),
                                             rhs=c2walk(0, 0, 0, pair=PYS),
                                             start=True, stop=False, perf_mode=DR)
                            nc.tensor.matmul(ps[:], lhsT=w2pair(1),
                                             rhs=c2walk(0, 0, 1, pair=PYS),
                                             start=False, stop=False, perf_mode=DR)
                            nc.tensor.matmul(ps[:], lhsT=w2pair(2),
                                             rhs=c2walk(0, 1, 0, pair=1),
                                             start=False, stop=False, perf_mode=DR)
                        else:
                            for dy, (py, yo) in enumerate([(0, 0), (1, 0), (0, 1)]):
                                for x0, wsel in ((0, w2a_s), (1, w2b_s)):
                                    nc.tensor.matmul(ps[:],
                                                     lhsT=wsel[:, dy * 128:(dy + 1) * 128],
                                                     rhs=c2walk(py, yo, x0),
                                                     start=(dy == 0 and x0 == 0),
                                                     stop=False)
                        # gC taps (K=64, x=2j2+2): full walks; the j2=15
                        # column wrongly reads f1[py, y+1, 0] instead of the
                        # zero pad -- negated-weight matmuls accumulate the
                        # cancellation into c2f, added into the j2=15 strip
                        # by the (otherwise idle) gpsimd engine
                        for dy, (py, yo) in enumerate([(0, 0), (1, 0), (0, 1)]):
                            nc.tensor.matmul(ps[:], lhsT=w2c_s[:, dy * 128:(dy + 1) * 128],
                                             rhs=c2walk(py, yo, 2)[0:64],
                                             start=False, stop=(dy == 2))
                        c2f = cfps.tile([128, 32], f32, tag="c2f")
                        for dy, (py, yo) in enumerate([(0, 0), (1, 0), (0, 1)]):
                            off = ib + py * PYS + yo * 32 + 32
                            rhs = bass.AP(fb.tensor, fb.offset + off,
                                          [fb.ap[0], [32, 32]])[0:64]
                            nc.tensor.matmul(c2f[:], lhsT=w2cn_s[:, dy * 128:(dy + 1) * 128],
                                             rhs=rhs, start=(dy == 0),
                                             stop=(dy == 2))
                        c2fs = f2p.tile([128, 32], bf16, tag="c2fs")
                        nc.scalar.activation(out=c2fs[:], in_=c2f[:], func=AF.Copy)
                        pst = ps[:]
                        strip15 = bass.AP(pst.tensor, pst.offset + 15,
                                          [pst.ap[0], [16, 32]])
                        nc.vector.tensor_tensor(out=strip15, in0=strip15,
                                                in1=c2fs[:], op=OP.add)
                        f2s = f2p.tile([128, 512], bf16, tag="f2s")
                        if i % 2 == 0:
                            nc.vector.tensor_scalar(out=f2s[:], in0=ps[:], scalar1=b2_s[:, 0:1],
                                                    scalar2=0.0, op0=OP.add, op1=OP.max)
                        else:
                            nc.scalar.activation(out=f2s[:], in_=ps[:], func=AF.Relu,
                                                 bias=b2_s[:, 0:1])
                        tp = tps.tile([128, 512], bf16, tag="tp")
                        for b in range(4):
                            nc.tensor.transpose(tp[:, b * 128:(b + 1) * 128],
                                                f2s[:, b * 128:(b + 1) * 128], ident_s[:])
                        f2t = f2p.tile([128, 512], bf16, tag="f2t")
                        if i % 2 == 0:
                            nc.scalar.activation(out=f2t[:], in_=tp[:], func=AF.Copy)
                        else:
                            nc.vector.tensor_copy(out=f2t[:], in_=tp[:])
                        # RoIAlign as matmul: pool_ps[c, n*16+pt] = sum over
                        # pixel chunks (b, px) of f2t-slice^T @ wroi-slice
                        img = g * IMG_GRP + i
                        pool_ps = pps.tile([64, 96], f32, tag="pool")
                        for c in range(8):
                            b, px = divmod(c, 2)
                            nc.tensor.matmul(
                                pool_ps[:],
                                lhsT=f2t[:, b * 128 + px * 64:b * 128 + px * 64 + 64],
                                rhs=wr[:, c * 96:c * 96 + 96],
                                start=(c == 0), stop=(c == 7))
                        if i % 2 == 0:
                            nc.scalar.activation(out=poolT[:, img * 96:(img + 1) * 96],
                                                 in_=pool_ps[:], func=AF.Copy)
                        else:
                            nc.vector.tensor_copy(out=poolT[:, img * 96:(img + 1) * 96],
                                                  in_=pool_ps[:])
                    if not getattr(lw, 'done', False):
                        late_chunks[g]()
                        if g == NGRP - 1:
                            lw.done = True
            if stage == 'conv':
                return

            # ================= fc0 + emb + red =================
            with ExitStack() as gx:
                ops = gx.enter_context(tc.tile_pool(name="ops", bufs=2, space="PSUM"))

                obj = sp.tile([128, 384], bf16, tag="obj")
                pview = poolT[:].rearrange("p (r t) -> p t r", t=16)
                for m2 in range(2):
                    ps = ops.tile([128, 192], f32, tag="obj")
                    for pt_i in range(16):
                        nc.tensor.matmul(ps[:], lhsT=lw.fc0t_s[:, pt_i * 256 + m2 * 128:
                                                            pt_i * 256 + m2 * 128 + 128],
                                         rhs=pview[:, pt_i, :],
                                         start=(pt_i == 0), stop=(pt_i == 15))
                    nc.scalar.activation(out=obj[:, m2 * 192:(m2 + 1) * 192], in_=ps[:],
                                         func=AF.Relu, bias=lw.fc0b_s[:, m2:m2 + 1])
                emb1 = sp.tile([128, 384], bf16, tag="emb1")
                for m2 in range(2):
                    ps = ops.tile([128, 192], f32, tag="emb")
                    nc.tensor.matmul(ps[:], lhsT=lw.fc0ct_s[:, m2 * 128:(m2 + 1) * 128],
                                     rhs=lw.coor_s[:], start=True, stop=True)
                    nc.scalar.activation(out=emb1[:, m2 * 192:(m2 + 1) * 192], in_=ps[:],
                                         func=AF.Relu, bias=lw.fc0cb_s[:, m2:m2 + 1])
                emb2 = sp.tile([128, 384], bf16, tag="emb2")
                for m2 in range(2):
                    ps = ops.tile([128, 192], f32, tag="emb")
                    for kc in range(2):
                        nc.tensor.matmul(ps[:], lhsT=lw.fc1ct_s[:, kc * 256 + m2 * 128:
                                                             kc * 256 + m2 * 128 + 128],
                                         rhs=emb1[:, kc * 192:(kc + 1) * 192],
                                         start=(kc == 0), stop=(kc == 1))
                    nc.scalar.activation(out=emb2[:, m2 * 192:(m2 + 1) * 192], in_=ps[:],
                                         func=AF.Relu, bias=lw.fc1cb_s[:, m2:m2 + 1])
                o2 = sp.tile([128, 384], bf16, tag="o2")
                for m2 in range(2):
                    ps = ops.tile([128, 192], f32, tag="o2")
                    for kc in range(2):
                        nc.tensor.matmul(ps[:], lhsT=lw.redoT_s[:, kc * 256 + m2 * 128:
                                                             kc * 256 + m2 * 128 + 128],
                                         rhs=obj[:, kc * 192:(kc + 1) * 192],
                                         start=(kc == 0), stop=False)
                    for kc in range(2):
                        nc.tensor.matmul(ps[:], lhsT=lw.redeT_s[:, kc * 256 + m2 * 128:
                                                             kc * 256 + m2 * 128 + 128],
                                         rhs=emb2[:, kc * 192:(kc + 1) * 192],
                                         start=False, stop=(kc == 1))
                    nc.scalar.activation(out=o2[:, m2 * 192:(m2 + 1) * 192], in_=ps[:],
                                         func=AF.Relu, bias=lw.redb_s[:, m2:m2 + 1])
                # initial states: s_m [128, 96] cols m2*48 + b*6 + n  <- o2 cols m2*192 + b*24 + m*6 + n
                o2v = o2[:].rearrange("p (m2 b t n) -> p m2 b t n", m2=2, b=8, t=4)
                for m in range(4):
                    nc.vector.tensor_copy(
                        out=st[m][:].rearrange("p (m2 b n) -> p m2 b n", m2=2, b=8),
                        in_=o2v[:, :, :, m, :])

            if stage.startswith('gather'):
                return

            # ================= GNN rollouts =================
            with ExitStack() as rx:
                gps = rx.enter_context(tc.tile_pool(name="gps", bufs=4, space="PSUM"))
                vps = rx.enter_context(tc.tile_pool(name="vps", bufs=2, space="PSUM"))
                sps = rx.enter_context(tc.tile_pool(name="sps", bufs=2, space="PSUM"))
                hb = rx.enter_context(tc.tile_pool(name="hbuf", bufs=3))

                def emit_uv(rr, ks):
                    # uv_ps rows 0-47 = u+rb = s@Wl^T + rb (rb via K=1 matmul),
                    # rows 64-111 = v = s@Wr^T; contiguous accumulation group
                    # per partition region; copies alternate DVE/Act
                    for k in ks:
                        s = st[rr + k]
                        uv_ps = vps.tile([112, 256], f32, tag="v")
                        # zero rows 32-63 first (write base must be 0/32/64; the
                        # u matmuls below re-cover 32-47 with real data).
                        # skip_group_check: self-contained start+stop write
                        # whose partition-offset aliases CoreSim's zero-region
                        # tracker against the row-0/row-64 groups
                        nc.tensor.matmul(uv_ps[32:64, :], lhsT=lw.ones2_s[0:1, 0:32],
                                         rhs=lw.zrow_s[:], start=True, stop=True,
                                         skip_group_check=True)
                        for kc in range(2):
                            nc.tensor.matmul(uv_ps[0:48, :], lhsT=s[:, kc * 48:kc * 48 + 48],
                                             rhs=lw.gWlT_s[:, k * 512 + kc * 256:
                                                        k * 512 + (kc + 1) * 256],
                                             start=(kc == 0), stop=False)
                        nc.tensor.matmul(uv_ps[0:48, :], lhsT=lw.ones2_s[0:1, :],
                                         rhs=lw.rbT_s[:, k * 256:(k + 1) * 256],
                                         start=False, stop=True)
                        for kc in range(2):
                            nc.tensor.matmul(uv_ps[64:112, :], lhsT=s[:, kc * 48:kc * 48 + 48],
                                             rhs=lw.gWrT_s[:, k * 512 + kc * 256:
                                                        k * 512 + (kc + 1) * 256],
                                             start=(kc == 0), stop=(kc == 1),
                                             skip_group_check=True)
                        if k % 2 == 0:
                            nc.vector.tensor_copy(out=uvt[k][:], in_=uv_ps[:])
                        else:
                            nc.scalar.activation(out=uvt[k][:], in_=uv_ps[:], func=AF.Copy)

                def emit_x(rr, k):
                    m = rr + k
                    s = st[rr + k]
                    # x = rel + deg*(u+rb) + self-dynamics; one contiguous psum
                    # accumulation group per half (interleaved groups in one
                    # bank mis-accumulate): rel first, then sd matmuls
                    x_ps = gps.tile([128, 96], f32, tag="g")
                    for m2 in range(2):
                        nc.tensor.matmul(x_ps[:, m2 * 48:m2 * 48 + 48],
                                         lhsT=uvt[k][:, m2 * 128:(m2 + 1) * 128],
                                         rhs=mdd_t[m][:], start=True, stop=False)
                        for kc in range(2):
                            lo = k * 512 + kc * 256 + m2 * 128
                            nc.tensor.matmul(x_ps[:, m2 * 48:m2 * 48 + 48],
                                             lhsT=lw.gswT_s[:, lo:lo + 128],
                                             rhs=s[:, kc * 48:kc * 48 + 48],
                                             start=False, stop=False)
                        nc.tensor.matmul(x_ps[:, m2 * 48:m2 * 48 + 48],
                                         lhsT=lw.gbiasT_s[:, k * 256 + m2 * 128:
                                                          k * 256 + m2 * 128 + 128],
                                         rhs=lw.ones2_s[0:1, :],
                                         start=False, stop=True)
                    return x_ps

                def evac_plain(ps_t, k, tag, relu):
                    # bias already accumulated on PE; single whole-tile copy,
                    # alternating engines by head parity
                    sb = hb.tile([128, 96], bf16, tag=f"{tag}{k}")
                    if relu:
                        if k % 2 == 0:
                            nc.scalar.activation(out=sb[:], in_=ps_t[:],
                                                 func=AF.Relu, bias=0.0)
                        else:
                            nc.vector.tensor_scalar(out=sb[:], in0=ps_t[:],
                                                    scalar1=0.0, scalar2=None,
                                                    op0=OP.max)
                    else:
                        if k % 2 == 0:
                            nc.vector.tensor_copy(out=sb[:], in_=ps_t[:])
                        else:
                            nc.scalar.activation(out=sb[:], in_=ps_t[:], func=AF.Copy)
                    return sb

                def emit_a(k, x_sbs):
                    a_ps = gps.tile([128, 96], f32, tag="g")
                    for m2 in range(2):
                        for kc in range(2):
                            lo = k * 512 + kc * 256 + m2 * 128
                            nc.tensor.matmul(a_ps[:, m2 * 48:m2 * 48 + 48],
                                             lhsT=lw.gawT_s[:, lo:lo + 128],
                                             rhs=x_sbs[k][:, kc * 48:kc * 48 + 48],
                                             start=(kc == 0), stop=False)
                        nc.tensor.matmul(a_ps[:, m2 * 48:m2 * 48 + 48],
                                         lhsT=lw.gbiasT_s[:, 1024 + k * 256 + m2 * 128:
                                                          1024 + k * 256 + m2 * 128 + 128],
                                         rhs=lw.ones2_s[0:1, :],
                                         start=False, stop=True)
                    return a_ps

                def emit_o(rr, k, a_sbs):
                    s = st[rr + k]
                    o_ps = gps.tile([128, 96], f32, tag="g")
                    for m2 in range(2):
                        for kc in range(2):
                            lo = k * 512 + kc * 256 + m2 * 128
                            nc.tensor.matmul(o_ps[:, m2 * 48:m2 * 48 + 48],
                                             lhsT=lw.gowaT_s[:, lo:lo + 128],
                                             rhs=a_sbs[k][:, kc * 48:kc * 48 + 48],
                                             start=(kc == 0), stop=False)
                            nc.tensor.matmul(o_ps[:, m2 * 48:m2 * 48 + 48],
                                             lhsT=lw.gowsT_s[:, lo:lo + 128],
                                             rhs=s[:, kc * 48:kc * 48 + 48],
                                             start=False, stop=False)
                        nc.tensor.matmul(o_ps[:, m2 * 48:m2 * 48 + 48],
                                         lhsT=lw.gbiasT_s[:, 2048 + k * 256 + m2 * 128:
                                                          2048 + k * 256 + m2 * 128 + 128],
                                         rhs=lw.ones2_s[0:1, :],
                                         start=False, stop=True)
                    return o_ps

                emit_uv(0, range(4))
                tail = [None]
                for rr in range(8):
                    # stage-major emission; head 3 trails heads 0-2 by one
                    # stage so the previous rollout's dec+mask chain (emitted
                    # between, as `tail`) can resolve off the PE critical path
                    x_pss, x_sbs, a_pss, a_sbs, o_pss, cs = [], [], [], [], [], []
                    for k in range(3):
                        x_pss.append(emit_x(rr, k))
                    for k in range(3):
                        x_sbs.append(evac_plain(x_pss[k], k, "x", False))
                    if tail[0] is not None:
                        tail[0]()
                        tail[0] = None
                    for k in range(3):
                        a_pss.append(emit_a(k, x_sbs))
                    x_pss.append(emit_x(rr, 3))
                    x_sbs.append(evac_plain(x_pss[3], 3, "x", False))
                    for k in range(3):
                        a_sbs.append(evac_plain(a_pss[k], k, "a", True))
                    for k in range(3):
                        o_pss.append(emit_o(rr, k, a_sbs))
                    a_pss.append(emit_a(3, x_sbs))
                    a_sbs.append(evac_plain(a_pss[3], 3, "a", True))
                    for k in range(3):
                        cs.append(evac_plain(o_pss[k], k, "cs", True))
                    o_pss.append(emit_o(rr, 3, a_sbs))
                    cs.append(evac_plain(o_pss[3], 3, "cs", True))
                    if dump and rr == 0:
                        for k in range(4):
                            nc.sync.dma_start(dbg_uvt[:, k * 256:(k + 1) * 256], uvt[k][:])
                            nc.sync.dma_start(dbg_x[:, k * 96:(k + 1) * 96], x_sbs[k][:])
                            nc.sync.dma_start(dbg_a[:, k * 96:(k + 1) * 96], a_sbs[k][:])
                    if rr < 7:
                        emit_uv(rr + 1, range(3))
                    g_ps = gps.tile([128, 96], f32, tag="g")
                    for m2 in range(2):
                        n = 0
                        for k in range(4):
                            for kc in range(2):
                                lo = (k * 2 + kc) * 256 + m2 * 128
                                nc.tensor.matmul(g_ps[:, m2 * 48:m2 * 48 + 48],
                                                 lhsT=lw.aggT_s[:, lo:lo + 128],
                                                 rhs=cs[k][:, kc * 48:kc * 48 + 48],
                                                 start=(n == 0), stop=False)
                                n += 1
                        nc.tensor.matmul(g_ps[:, m2 * 48:m2 * 48 + 48],
                                         lhsT=lw.aggbT_s[:, m2 * 128:(m2 + 1) * 128],
                                         rhs=lw.ones2_s[0:1, :],
                                         start=False, stop=True)
                    if dump and rr == 0:
                        for k in range(4):
                            nc.sync.dma_start(dbg_cs[:, k * 96:(k + 1) * 96], cs[k][:])
                            nc.sync.dma_start(dbg_mdd[:, k * 48:(k + 1) * 48], mdd_t[k][:])
                    s_new = st[rr + 4]
                    nc.vector.tensor_copy(out=s_new[:], in_=g_ps[:])
                    if rr < 7:
                        emit_uv(rr + 1, [3])

                    def make_tail(rr, s_new):
                        def tail_fn():
                            d_ps = sps.tile([4, 48], f32, tag="s")
                            for kc in range(2):
                                nc.tensor.matmul(d_ps[:], lhsT=lw.decT_s[:, kc * 4:kc * 4 + 4],
                                                 rhs=s_new[:, kc * 48:kc * 48 + 48],
                                                 start=(kc == 0), stop=(kc == 1))
                            bbv = bbox_sb[:].rearrange("f (b q) -> f b q", b=8)[:, :, rr * 6:rr * 6 + 6]
                            nc.vector.tensor_scalar(out=bbv, in0=d_ps[:],
                                                    scalar1=lw.decb4_s[:, 0:1],
                                                    scalar2=None, op0=OP.add)
                            if rr >= 7:
                                return
                            m = rr + 4
                            d2_ps = sps.tile([2, 48], f32, tag="s")
                            for kc in range(2):
                                nc.tensor.matmul(d2_ps[:], lhsT=lw.decT_s[:, kc * 4 + 2:kc * 4 + 4],
                                                 rhs=s_new[:, kc * 48:kc * 48 + 48],
                                                 start=(kc == 0), stop=(kc == 1))
                            # coorb / -2*coorb / coorb^2 all read d2_ps directly
                            # (coorb is only needed as a matmul lhsT)
                            coorb = hb.tile([2, 48], bf16, tag="coorb")
                            nc.vector.tensor_scalar(out=coorb[:], in0=d2_ps[:],
                                                    scalar1=lw.decb4_s[0:2, 1:2],
                                                    scalar2=None, op0=OP.add)
                            cm2 = hb.tile([2, 48], bf16, tag="cm2")
                            nc.vector.tensor_scalar(out=cm2[:], in0=d2_ps[:],
                                                    scalar1=lw.decb4_s[0:2, 1:2],
                                                    scalar2=-2.0, op0=OP.add,
                                                    op1=OP.mult)
                            sq = hb.tile([2, 48], bf16, tag="sq")
                            nc.scalar.activation(out=sq[:], in_=d2_ps[:], func=AF.Square,
                                                 bias=lw.decb4_s[0:2, 1:2])
                            m_ps = sps.tile([112, 48], f32, tag="s")
                            nc.tensor.matmul(m_ps[64:112, :], lhsT=coorb[:], rhs=cm2[:],
                                             start=True, stop=False,
                                             skip_group_check=True)
                            nc.tensor.matmul(m_ps[64:112, :], lhsT=sq[:], rhs=lw.ones2_s[:],
                                             start=False, stop=False,
                                             skip_group_check=True)
                            nc.tensor.matmul(m_ps[64:112, :], lhsT=lw.ones2_s[:], rhs=sq[:],
                                             start=False, stop=True,
                                             skip_group_check=True)
                            nc.vector.tensor_tensor(out=mdd_t[m][64:112, :], in0=m_ps[64:112, :],
                                                    in1=lw.Tm_s[64:112, :], op=OP.is_le)
                            dd_ps = sps.tile([128, 48], f32, tag="s")
                            nc.tensor.matmul(dd_ps[:], lhsT=lw.ones48_s[64:112, :],
                                             rhs=mdd_t[m][64:112, :], start=True, stop=True)
                            nc.vector.tensor_tensor(out=mdd_t[m][0:48, :], in0=lw.eye48_s[:],
                                                    in1=dd_ps[0:48, :], op=OP.mult)
                        return tail_fn
                    tail[0] = make_tail(rr, s_new)
                tail[0]()
                tail[0] = None
                if dump:
                    for m in range(5):
                        nc.sync.dma_start(dbg_st[:, m * 96:(m + 1) * 96], st[m][:])
        for _rep in range(nrep):
            stages()
        if stage != 'full':
            nc.gpsimd.memset(bbox_sb[:], 0.0)
        nc.sync.dma_start(
            out[:].rearrange("b rr n f -> f (b rr n)"), bbox_sb[:])
    return nc


_NC = None

def _get_nc():
    global _NC
    if _NC is None:
        nc = bass.Bass()
        build(nc)
        split_drain_waits(nc)
        _NC = nc
    return _NC


def kernel(**inputs):
    nc = _get_nc()
    inputs = {k: np.asarray(v) for k, v in inputs.items()}
    maps = [make_core_inputs(inputs, s) for s in range(NCORE)]
    res = run_bass_kernel_spmd(nc, maps, core_ids=list(range(NCORE)))
    out = np.concatenate([res.results[s]["bbox_out"] for s in range(NCORE)], 0)
    return out.astype(np.float32)



# revision 69
# speedup vs baseline: 1.1823x; 1.0602x over previous
"""Trainium2 Bass kernel for nn_Net_63754494542044.

Data-parallel over 8 NeuronCores (8 B-samples each). Host pre-packs
conv1 im2col / conv weights / RoIAlign grid tables; device runs
conv1 -> conv2 -> RoIAlign gather+bilinear -> fc0/emb/red -> 8 GNN rollouts.
"""
import sys
sys.path.insert(0, '/opt/trn_rl_repo')
import numpy as np
from contextlib import ExitStack
import concourse.bass as bass
import concourse.tile as tile
from concourse import mybir
from concourse.bass_utils import run_bass_kernel_spmd

# Walrus wait-slot limits: CTRL-encoded (Drain/NoOp) = 1; others appear
# limited too on this build -- split conservatively.
def split_drain_waits(nc, max_waits=1, max_waits_other=1):
    for fn in nc.m.functions:
        for bb in fn.blocks:
            insts = bb.instructions
            i = 0
            while i < len(insts):
                inst = insts[i]
                si = getattr(inst, 'sync_info', None)
                lim = max_waits if isinstance(inst, (mybir.InstDrain, mybir.InstNoOp)) else max_waits_other
                if si is not None and si.on_wait and len(si.on_wait) > lim:
                    waits = list(si.on_wait)
                    keep = waits[-lim:]
                    extra = waits[:-lim]
                    new_nops = []
                    for k in range(0, len(extra), max_waits):
                        chunk = extra[k:k + max_waits]
                        nop = mybir.InstNoOp(
                            name=nc.get_next_instruction_name(),
                            engine=inst.engine,
                        )
                        nop.sync_info = mybir.SyncInfo(on_wait=chunk, on_update=[])
                        nc.register_instruction(nop)
                        new_nops.append(nop)
                    inst.sync_info = mybir.SyncInfo(on_wait=keep, on_update=list(si.on_update))
                    insts[i:i] = new_nops
                    i += len(new_nops)
                i += 1


import os
FP8_CONV = os.environ.get('BASSK_FP8', '1') == '1'

B, T, N = 64, 4, 6
IMG, CIN = 128, 3
VE, D, P = 64, 256, 4
SCALE = 0.25
NCORE = 8
BC = B // NCORE          # 8 samples per core
NIMG = BC * T            # 32 images per core
NROI = BC * T * N        # 192 rois per core
NROW = BC * N            # 48 gnn rows per core
NPT = NROI * 16          # 3072 sample points per core
NG = 24                  # gather groups


# ---------------- conv1 im2col (host) ----------------
# conv1: 3->64, 3x3, stride2, SAME on 128x128 -> 64x64.
# 2-px-packed output: out pair (oy, j) covers ox = 2j, 2j+1.
# K=45 rows: (rowtap rt in 0..2) x (coltap ct in 0..4) x (ci in 0..2)
#   input row for out oy: rt0: 2*oy-1, rt1: 2*oy, rt2: 2*oy+1
#   input col for out pair j: ct: 4j-1, 4j, 4j+1, 4j+2, 4j+3
def conv1_im2col_host(x):  # x [nimg, 3, 128, 128] fp32
    nimg = x.shape[0]
    xp = np.pad(x, ((0, 0), (0, 0), (0, 1), (0, 1)))  # SAME stride2: pad bottom/right only
    cols = np.empty((45, nimg, 64, 32), np.float32)
    k = 0
    for rt in range(3):
        for ct in range(5):
            for ci in range(3):
                # row = 2*oy + rt ; col = 4*j + ct
                cols[k] = xp[:, ci, rt:rt + 127:2, ct:ct + 125:4]
                k += 1
    return cols  # [45, nimg, 64, 32]


def conv1_weights_host(w_conv1):  # [64, 3, 3, 3]
    # W2 [45, 128]: col m = px*64 + oc ... out(oy, 2j+px) uses taps:
    #   orig tap (dy, dx): input row 2oy+dy-1 -> rt = dy ; input col 2(2j+px)+dx-1 = 4j + (2px+dx-1) -> ct = 2px+dx-1
    W2 = np.zeros((45, 128), np.float32)
    for px in range(2):
        for oc in range(64):
            m = px * 64 + oc
            for dy in range(3):
                for dx in range(3):
                    ct = 2 * px + dx
                    assert 0 <= ct <= 4  # input col = 4j + ct (no left pad)
                    for ci in range(3):
                        W2[(dy * 5 + ct) * 3 + ci, m] = w_conv1[oc, ci, dy, dx]
    return W2


def conv1_host(x, w_conv1, b_conv1):
    """Mirror of device conv1: returns feat1 [nimg, 64, 64, 64] (pre-relu + bias)."""
    cols = conv1_im2col_host(x)          # [45, nimg, 64, 32]
    W2 = conv1_weights_host(w_conv1)     # [45, 128]
    out = np.einsum('kf,kc->cf', cols.reshape(45, -1), W2)  # [128, nimg*64*32]
    out = out.reshape(2, 64, -1, 64, 32)  # [px, oc, img, oy, j]
    feat1 = np.empty((x.shape[0], 64, 64, 64), np.float32)
    feat1[..., 0::2] = np.transpose(out[0], (1, 0, 2, 3))
    feat1[..., 1::2] = np.transpose(out[1], (1, 0, 2, 3))
    feat1 += b_conv1[None, :, None, None]
    return feat1


# ---------------- conv2 weights (host) ----------------
# feat1_ph partitions: (px_in*64 + ci'), free (img, py, Y, X) halo X,Y in -1..31.
# conv2 out pair (oy2, j2): outs o1=2*j2, o2=2*j2+1 ; M col = pxo*64 + oc.
# 9 matmuls: rowtap r in {py0[Y], py1[Y-1], py1[Y]} x colgrp g in {X=j2 pair(K128), X=j2+?...}
# col groups: g0: pair (px0[Xa], px1[Xa]) Xa = j2? ... define by original dx:
#   out ox2: input x = 2*ox2 + dx - 1
#   for o1=2j2: x = 4j2-1, 4j2, 4j2+1 -> (px,X): (1, 2j2-1), (0, 2j2), (1, 2j2)
#   for o2=2j2+1: x = 4j2+1, 4j2+2, 4j2+3 -> (1, 2j2), (0, 2j2+1), (1, 2j2+1)
# X taps: px0: {2j2, 2j2+1} ; px1: {2j2-1, 2j2, 2j2+1}
# col groups (relative X offset from base 2j2):
#   gA: K128 = (px0[2j2], px1[2j2])        -> X offset 0, both phases
#   gB: K128 = (px0[2j2+1], px1[2j2+1])    -> X offset +1, both phases
#   gC: K64  = px1[2j2-1]                  -> X offset -1, px1 only
# rowtaps r (input y = 2*oy2 + dy - 1):
#   dy0: y = 2oy2-1 -> (py1, Y=oy2-1) ; dy1: y=2oy2 -> (py0, Y=oy2) ; dy2: y=2oy2+1 -> (py1, Y=oy2)
def conv2_weights_host(w_conv2):  # [64, 64, 3, 3]
    # Wb[r][g]: gA/gB: [128, 128] (partition = pxi*64+ci), gC: [64, 128]
    # dy maps to rowtap r directly (r=0: dy=0 ; r=1: dy=1 ; r=2: dy=2)
    Wb = [[np.zeros((128, 128), np.float32) for _ in range(2)] + [np.zeros((64, 128), np.float32)]
          for _ in range(3)]
    for pxo in range(2):          # which output in the pair (o = 2j2+pxo)
        for oc in range(64):
            m = pxo * 64 + oc
            for dy in range(3):
                for dx in range(3):
                    x_off = 2 * pxo + dx     # input x = 4j2 + x_off, x_off in 0..4
                    pxi = x_off % 2
                    Xrel = x_off // 2        # in {0, 1, 2}
                    for ci in range(64):
                        if Xrel < 2:
                            Wb[dy][Xrel][pxi * 64 + ci, m] += w_conv2[oc, ci, dy, dx]
                        else:
                            assert pxi == 0
                            Wb[dy][2][ci, m] += w_conv2[oc, ci, dy, dx]
    return Wb


def conv2_host(feat1r, w_conv2, b_conv2):
    """feat1r: relu'd feat1 [nimg, 64, 64, 64]. Returns feat2 [nimg, 64, 32, 32] pre-relu."""
    nimg = feat1r.shape[0]
    # build feat1_ph with halo: [128 part (pxi*64+ci), img, py, Y(-1..31), X(-1..31)]
    ph = np.zeros((128, nimg, 2, 33, 33), np.float32)  # halo at Y=32, X=32
    f = feat1r  # [img, ci, y, x]
    for pxi in range(2):
        for py in range(2):
            ph[pxi * 64:pxi * 64 + 64, :, py, :32, :32] = np.transpose(
                f[:, :, py::2, pxi::2], (1, 0, 2, 3))
    Wb = conv2_weights_host(w_conv2)
    out = np.zeros((128, nimg, 32, 16), np.float32)  # [(pxo,oc), img, oy2, j2]
    # rowtap dy: input y = 2*oy2 + dy -> (py = dy&1, Y = oy2 + dy//2)
    for dy in range(3):
        py, Yoff = dy % 2, dy // 2
        for g in range(3):
            W = Wb[dy][g]
            Ysl = slice(Yoff, Yoff + 32)
            Xidx = g + 2 * np.arange(16)   # X = 2*j2 + Xrel ... stored X index = that
            rhs = ph[:, :, py, Ysl, :][:, :, :, Xidx]  # [128 or .., img, 32, 16]
            if g == 2:
                rhs = rhs[:64]
            out += np.einsum('km,kijx->mijx', W, rhs)
    feat2 = np.empty((nimg, 64, 32, 32), np.float32)
    feat2[..., 0::2] = np.transpose(out[:64], (1, 0, 2, 3))
    feat2[..., 1::2] = np.transpose(out[64:], (1, 0, 2, 3))
    return feat2 + b_conv2[None, :, None, None]


# ---------------- RoIAlign grid (host) ----------------
def roi_grid_host(rois):  # rois [NROI, 5] fp32 (batch-local; bidx = local img idx)
    """Returns idx int32 [NPT, 2] (row-gather indices, row=(img,y,j2) width 128),
    weights w4 [NPT, 4] fp32 (w00,w01,w10,w11 order: (y0x0, y0x1, y1x0, y1x1)),
    parity [NPT] (x0&1)."""
    nroi = rois.shape[0]
    W = H = 32
    x1 = rois[:, 1] * SCALE; y1 = rois[:, 2] * SCALE
    x2 = rois[:, 3] * SCALE; y2 = rois[:, 4] * SCALE
    bw = np.maximum(x2 - x1, 1.0) / P
    bh = np.maximum(y2 - y1, 1.0) / P
    grid = np.arange(P, dtype=np.float32) + 0.5
    sx = x1[:, None, None] + bw[:, None, None] * grid[None, None, :]   # [R, P(py), P(px)]
    sy = y1[:, None, None] + bh[:, None, None] * grid[None, :, None]
    sx = np.broadcast_to(sx, (nroi, P, P)).reshape(-1)
    sy = np.broadcast_to(sy, (nroi, P, P)).reshape(-1)
    x0f = np.clip(np.floor(sx), 0, W - 1)
    y0f = np.clip(np.floor(sy), 0, H - 1)
    lx = np.clip(sx - x0f, 0.0, 1.0)
    ly = np.clip(sy - y0f, 0.0, 1.0)
    # clamp x0 to <= 30 adjusting lx (exact when sx>=31: both corners read col 31)
    x0 = x0f.astype(np.int32); y0 = y0f.astype(np.int32)
    hi = x0 >= 31
    x0 = np.where(hi, 30, x0); lx = np.where(hi, 1.0, lx).astype(np.float32)
    hiy = y0 >= 31
    y0 = np.where(hiy, 30, y0); ly = np.where(hiy, 1.0, ly).astype(np.float32)
    img = np.repeat(np.arange(nroi, dtype=np.int32) // N, 16)
    j2 = x0 >> 1
    par = (x0 & 1).astype(np.float32)
    idx0 = img * 512 + y0 * 16 + j2          # row idx (rows of 128 els)
    idx1 = idx0 + 16                          # y0+1 row
    w4 = np.stack([(1 - ly) * (1 - lx), (1 - ly) * lx, ly * (1 - lx), ly * lx], 1).astype(np.float32)
    return np.stack([idx0, idx1], 1).astype(np.int32), w4, par


def roi_wmat_host(rois):
    """RoIAlign as per-image matmul: sparse bilinear weights densified.
    Returns Wg [NIMG, 8, 128, 96] f32: for image i, chunk c = b*2+px
    (b = pair block 0..3, px = x parity), Wg[i, c, pair_local, n*16+pt] =
    bilinear weight of pixel (y, x) for point pt of roi n, where
    pair = y*16 + (x>>1) = b*128 + pair_local."""
    nroi = rois.shape[0]
    x1 = rois[:, 1] * SCALE; y1 = rois[:, 2] * SCALE
    x2 = rois[:, 3] * SCALE; y2 = rois[:, 4] * SCALE
    bw = np.maximum(x2 - x1, 1.0) / P
    bh = np.maximum(y2 - y1, 1.0) / P
    grid = np.arange(P, dtype=np.float32) + 0.5
    sx = x1[:, None, None] + bw[:, None, None] * grid[None, None, :]
    sy = y1[:, None, None] + bh[:, None, None] * grid[None, :, None]
    sx = np.broadcast_to(sx, (nroi, P, P)).reshape(-1)
    sy = np.broadcast_to(sy, (nroi, P, P)).reshape(-1)
    x0f = np.clip(np.floor(sx), 0, 31); y0f = np.clip(np.floor(sy), 0, 31)
    lx = np.clip(sx - x0f, 0.0, 1.0).astype(np.float32)
    ly = np.clip(sy - y0f, 0.0, 1.0).astype(np.float32)
    x0 = x0f.astype(np.int64); y0 = y0f.astype(np.int64)
    x1i = np.minimum(x0 + 1, 31); y1i = np.minimum(y0 + 1, 31)
    img = np.arange(nroi).repeat(16) // N
    col = (np.arange(nroi) % N).repeat(16) * 16 + np.tile(np.arange(16), nroi)
    Wg = np.zeros((NIMG, 8, 128, 96), np.float32)
    flat = Wg.reshape(-1)
    for w, yy, xx in (((1 - ly) * (1 - lx), y0, x0), ((1 - ly) * lx, y0, x1i),
                      (ly * (1 - lx), y1i, x0), (ly * lx, y1i, x1i)):
        pair = yy * 16 + (xx >> 1)
        c = (pair >> 7) * 2 + (xx & 1)
        idxf = ((img * 8 + c) * 128 + (pair & 127)) * 96 + col
        np.add.at(flat, idxf, w)
    return Wg


def roi_align_host(feat2r, rois):
    """Mirror of device pool-matmul -> pooled [NPT, 64] pt-major."""
    Wg = roi_wmat_host(rois)                       # [NIMG, 8, 128, 96]
    # F2c[i, c, pl, ch] = feat2r[i, ch, y, x], c = (y//8)*2 + (x&1),
    # pl = (y%8)*16 + (x>>1)
    f = feat2r.reshape(NIMG, 64, 4, 8, 16, 2)       # [i, ch, b, y8, j2, px]
    F2c = np.transpose(f, (0, 2, 5, 3, 4, 1)).reshape(NIMG, 4, 2, 128, 64)
    F2c = F2c.reshape(NIMG, 8, 128, 64)             # chunk order (b, px) ✓
    pooled = np.einsum('icpn,icpm->inm', Wg, F2c)   # [i, 96, 64]
    return pooled.reshape(NPT, 64)


# ---------------- GNN (host mirror of device algebra) ----------------
def mask_host(coor, r):
    """coor [BC, N, 2], r [BC, N] -> bigmask [NROW, NROW] fp32 block-diag, deg [NROW]."""
    bm = np.zeros((NROW, NROW), np.float32)
    for b in range(BC):
        d = np.linalg.norm(coor[b][:, None, :] - coor[b][None, :, :], axis=-1)
        m = (d <= (r[b][:, None] + r[b][None, :])) & ~np.eye(N, dtype=bool)
        bm[b * N:(b + 1) * N, b * N:(b + 1) * N] = m
    return bm, bm.sum(1)


def internet_host(s, bm, deg, p):
    """s [NROW, D] fp32 row-major; bm [NROW,NROW]; p = (sw,sb,rw,rb,aw,ab,ow,ob)."""
    sw, sb, rw, rb, aw, ab, ow, ob = p
    Wl, Wr = rw[:, :D], rw[:, D:]
    self_d = s @ sw.T + sb
    u = s @ Wl.T + rb
    v = s @ Wr.T
    rel = deg[:, None] * u + bm @ v
    a = np.maximum((self_d + rel) @ aw.T + ab, 0)
    return np.maximum(a @ ow[:, :D].T + s @ ow[:, D:].T + ob, 0)


def gnn_host(obj_t, src_coor, r, inputs):
    """obj_t [4][NROW, D] initial states; src_coor [BC, T, N, 2]; r [BC, N].
    Returns bboxes [BC, 8, N, 4]."""
    states = list(obj_t)
    masks = [mask_host(src_coor[:, t], r) for t in range(4)]
    num_rollouts = int(inputs['num_rollouts'])
    out = []
    for rr in range(num_rollouts):
        cs = []
        for k in range(4):
            p = (inputs['g_self_w'][k], inputs['g_self_b'][k], inputs['g_rel_w'][k],
                 inputs['g_rel_b'][k], inputs['g_aff_w'][k], inputs['g_aff_b'][k],
                 inputs['g_out_w'][k], inputs['g_out_b'][k])
            bm, deg = masks[k]
            cs.append(internet_host(states[k], bm, deg, p))
        s = np.concatenate(cs, -1) @ inputs['agg_w'].T + inputs['agg_b']
        bbox = s @ inputs['dec_w'].T + inputs['dec_b']          # [NROW, 4]
        out.append(bbox.reshape(BC, N, 4))
        states = states[1:] + [s]
        coor = bbox[:, 2:].reshape(BC, N, 2)
        masks = masks[1:] + [mask_host(coor, r)]
    return np.stack(out, 1)


def full_host(inputs, shard):
    """Complete per-core mirror (fp32). shard = B-slice index."""
    sl = slice(shard * BC, (shard + 1) * BC)
    x = inputs['x'][sl].reshape(NIMG, CIN, IMG, IMG)
    rois = inputs['rois'][sl].reshape(NROI, 5)
    coor = inputs['src_coor_features'][sl]                      # [BC, T, N, 2]
    r = (((rois.reshape(BC, T, N, 5)[..., 4] - rois.reshape(BC, T, N, 5)[..., 2]) / 2
          + (rois.reshape(BC, T, N, 5)[..., 3] - rois.reshape(BC, T, N, 5)[..., 1]) / 2) / 2).mean(1)
    f1 = np.maximum(conv1_host(x, inputs['w_conv1'], inputs['b_conv1']), 0)
    f2 = np.maximum(conv2_host(f1, inputs['w_conv2'], inputs['b_conv2']), 0)
    pooled = roi_align_host(f2, rois)                           # [NPT, 64] pt-major
    # fc0: obj[row, d] = sum_{c,pt} pool[row, pt, c] * fc0_w[d, c*16+pt]
    pool_cp = pooled.reshape(NROI, 16, 64)
    Wp = inputs['fc0_w'].reshape(D, 64, 16)                     # [d, c, pt]
    obj = np.einsum('rpc,dcp->rd', pool_cp, Wp) + inputs['fc0_b']
    obj = np.maximum(obj, 0)                                    # [NROI, D] rows (b,t,n)
    emb = np.maximum(coor.reshape(NROI, 2) @ inputs['fc0c_w'].T + inputs['fc0c_b'], 0)
    emb = np.maximum(emb @ inputs['fc1c_w'].T + inputs['fc1c_b'], 0)
    o2 = np.maximum(obj @ inputs['red_w'][:, :D].T + emb @ inputs['red_w'][:, D:].T
                    + inputs['red_b'], 0)                       # [NROI, D]
    o2 = o2.reshape(BC, T, N, D)
    obj_t = [o2[:, t].reshape(NROW, D) for t in range(4)]
    return gnn_host(obj_t, coor, r, inputs)


# ---------------- device input packing ----------------
def make_core_inputs(inputs, shard):
    import ml_dtypes
    bf16 = ml_dtypes.bfloat16
    sl = slice(shard * BC, (shard + 1) * BC)
    x = np.asarray(inputs['x'][sl], np.float32).reshape(NIMG, CIN, IMG, IMG)
    rois = np.asarray(inputs['rois'][sl], np.float32).reshape(NROI, 5)
    coor = np.asarray(inputs['src_coor_features'][sl], np.float32)   # [BC,T,N,2]
    rr5 = rois.reshape(BC, T, N, 5)
    r = (((rr5[..., 4] - rr5[..., 2]) / 2 + (rr5[..., 3] - rr5[..., 1]) / 2) / 2).mean(1)

    fp8 = ml_dtypes.float8_e4m3
    cdt = fp8 if FP8_CONV else bf16
    d = {}
    cols = conv1_im2col_host(x).reshape(45, -1)       # [45, NIMG*64*32]
    if FP8_CONV:
        c46 = np.zeros((46, cols.shape[1]), np.float32)
        c46[:45] = cols
        # DoubleRow pair layout: row p holds tap p | tap 23+p side by side
        d['im2col45'] = np.concatenate([c46[:23], c46[23:]], 1).astype(fp8)
        w46 = np.zeros((46, 128), np.float32)
        w46[:45] = conv1_weights_host(np.asarray(inputs['w_conv1']))
        d['w1'] = np.concatenate([w46[:23], w46[23:]], 1).astype(fp8)
    else:
        d['im2col45'] = cols.astype(bf16)
        d['w1'] = conv1_weights_host(np.asarray(inputs['w_conv1'])).astype(bf16)
    b1 = np.asarray(inputs['b_conv1'], np.float32)
    d['b1'] = np.tile(b1, 2).reshape(128, 1).astype(np.float32)
    Wb = conv2_weights_host(np.asarray(inputs['w_conv2']))
    if FP8_CONV:
        # DoubleRow pairs: P0=[gA-dy0,gA-dy1], P1=[gB-dy0,gB-dy1],
        # P2=[gA-dy2,gB-dy2]; gC stays as 3 singles whose x-wrap at j2=15
        # is cancelled by small correction matmuls with negated weights
        d['w2p'] = np.stack([
            np.stack([Wb[0][0], Wb[1][0]], 1).reshape(128, 256),
            np.stack([Wb[0][1], Wb[1][1]], 1).reshape(128, 256),
            np.stack([Wb[2][0], Wb[2][1]], 1).reshape(128, 256)]).astype(cdt)
    else:
        d['w2a'] = np.stack([Wb[dy][0] for dy in range(3)]).astype(cdt)
        d['w2b'] = np.stack([Wb[dy][1] for dy in range(3)]).astype(cdt)
    d['w2c'] = np.stack([Wb[dy][2] for dy in range(3)]).astype(cdt)
    d['w2cn'] = np.stack([-Wb[dy][2] for dy in range(3)]).astype(cdt)
    b2 = np.asarray(inputs['b_conv2'], np.float32)
    d['b2'] = np.tile(b2, 2).reshape(128, 1).astype(np.float32)

    Wg = roi_wmat_host(rois)                          # [NIMG, 8, 128, 96]
    # device layout per group g: [128 pair_local, (img_local, chunk, pt) 6144]
    d['wroi'] = np.ascontiguousarray(
        Wg.reshape(NGRP, IMG_GRP, 8, 128, 96).transpose(0, 3, 1, 2, 4)
        .reshape(NGRP, 128, IMG_GRP * 8 * 96)).astype(bf16)

    fc0w = np.asarray(inputs['fc0_w'], np.float32).reshape(D, 64, 16)  # [d, c, pt]
    d['fc0t'] = np.ascontiguousarray(fc0w.transpose(2, 1, 0)).astype(bf16)  # [pt, c, d]
    d['fc0b'] = np.asarray(inputs['fc0_b'], np.float32).reshape(2, 128).T.copy()

    d['coor_fm'] = coor.reshape(NROI, 2).T.astype(bf16).copy()

    def t2(w):   # [256, K] -> [kc, 128, 256] lhsT chunks (w.T row-chunks)
        wT = np.ascontiguousarray(np.asarray(w, np.float32).T)       # [K, 256]
        K = wT.shape[0]
        return wT.reshape(K // 128, 128, 256).astype(bf16)

    def bcol(b):  # [256] -> [128, 2]
        return np.asarray(b, np.float32).reshape(2, 128).T.copy()

    d['fc0ct'] = np.asarray(inputs['fc0c_w'], np.float32).T.astype(bf16).copy()  # [2, 256]
    d['fc0cb'] = bcol(inputs['fc0c_b'])
    d['fc1ct'] = t2(inputs['fc1c_w'])
    d['fc1cb'] = bcol(inputs['fc1c_b'])
    redw = np.asarray(inputs['red_w'], np.float32)
    d['redoT'] = t2(redw[:, :D])
    d['redeT'] = t2(redw[:, D:])
    d['redb'] = bcol(inputs['red_b'])

    d['gswT'] = np.stack([t2(inputs['g_self_w'][k]) for k in range(4)])
    grw = np.asarray(inputs['g_rel_w'], np.float32)
    d['gWlT'] = np.stack([t2(grw[k][:, :D]) for k in range(4)])
    d['gWrT'] = np.stack([t2(grw[k][:, D:]) for k in range(4)])
    d['gawT'] = np.stack([t2(inputs['g_aff_w'][k]) for k in range(4)])
    gow = np.asarray(inputs['g_out_w'], np.float32)
    d['gowaT'] = np.stack([t2(gow[k][:, :D]) for k in range(4)])
    d['gowsT'] = np.stack([t2(gow[k][:, D:]) for k in range(4)])

    d['gbiasT'] = np.concatenate([
        np.asarray(inputs['g_self_b'], np.float32).reshape(-1),
        np.asarray(inputs['g_aff_b'], np.float32).reshape(-1),
        np.asarray(inputs['g_out_b'], np.float32).reshape(-1)]).reshape(1, 3072).astype(bf16)
    d['rbT'] = np.asarray(inputs['g_rel_b'], np.float32).reshape(1, 1024).astype(bf16)
    d['zrow'] = np.zeros((1, 256), bf16)
    d['aggT'] = t2(inputs['agg_w'])                    # [8, 128, 256]
    d['aggbT'] = np.asarray(inputs['agg_b'], np.float32).reshape(1, 256).astype(bf16)
    decw = np.asarray(inputs['dec_w'], np.float32)     # [4, 256]
    d['decT'] = decw.T.reshape(2, 128, 4).astype(bf16).copy()
    decb = np.asarray(inputs['dec_b'], np.float32).reshape(4)
    # col 0 = dec_b (for d_ps, partitions 0-3); col 1 rows 0-1 = dec_b[2:4]
    # (for d2_ps which lives on partitions 0-1)
    d['decb4'] = np.stack([decb, np.concatenate([decb[2:4], [0., 0.]])], 1)

    hmdds = []
    for m in range(4):
        bm, deg = mask_host(coor[:, m], r)
        mdd = np.zeros((112, NROW), np.float32)
        mdd[0:48] = np.diag(deg)
        mdd[64:112] = bm
        hmdds.append(mdd.astype(bf16))
    d['hmdd'] = np.stack(hmdds)
    Tmat = np.full((NROW, NROW), -1.0, np.float32)
    for b in range(BC):
        rs = (r[b][:, None] + r[b][None, :]) ** 2
        np.fill_diagonal(rs, -1.0)
        Tmat[b * N:(b + 1) * N, b * N:(b + 1) * N] = rs
    Tm112 = np.zeros((112, NROW), np.float32)
    Tm112[64:112] = Tmat
    d['Tm'] = Tm112
    d['ones48'] = np.ones((112, 128), bf16)
    d['ones2'] = np.ones((2, 48), bf16)
    d['ident'] = np.eye(128, dtype=bf16)
    d['eye48'] = np.eye(48, dtype=np.float32)
    return d


dt = mybir.dt
AF = mybir.ActivationFunctionType
OP = mybir.AluOpType

NIMG, NROI, NROW, NPT = 32, 192, 48, 3072
NG = 24            # gather groups (128 pts each)
IMG_GRP = 8        # images per conv group
NGRP = NIMG // IMG_GRP
IMGF = 2 * 33 * 32  # 2112 free els per img in feat1 (x tight, y has a zero row)


def build(nc: bass.Bass, dump=False, stage='full', nrep=1):
    f32, bf16, i32 = dt.float32, dt.bfloat16, dt.int32
    f8 = dt.float8e4 if FP8_CONV else dt.bfloat16

    def din(name, shape, d):
        return nc.dram_tensor(name, shape, d, kind="ExternalInput")

    if FP8_CONV:
        im2col = din("im2col45", [23, 131072], f8)
        w1 = din("w1", [23, 256], f8)
    else:
        im2col = din("im2col45", [45, 65536], f8)
        w1 = din("w1", [45, 128], f8)
    b1 = din("b1", [128, 1], f32)
    if FP8_CONV:
        w2p = din("w2p", [3, 128, 256], f8)
    else:
        w2a = din("w2a", [3, 128, 128], f8)
        w2b = din("w2b", [3, 128, 128], f8)
    w2c = din("w2c", [3, 64, 128], f8)
    w2cn = din("w2cn", [3, 64, 128], f8)
    b2 = din("b2", [128, 1], f32)
    wroi = din("wroi", [NGRP, 128, IMG_GRP * 8 * 96], bf16)
    fc0t = din("fc0t", [16, 64, 256], bf16)
    fc0b = din("fc0b", [128, 2], f32)
    coor = din("coor_fm", [2, 192], bf16)
    fc0ct = din("fc0ct", [2, 256], bf16)
    fc0cb = din("fc0cb", [128, 2], f32)
    fc1ct = din("fc1ct", [2, 128, 256], bf16)
    fc1cb = din("fc1cb", [128, 2], f32)
    redoT = din("redoT", [2, 128, 256], bf16)
    redeT = din("redeT", [2, 128, 256], bf16)
    redb = din("redb", [128, 2], f32)
    gswT = din("gswT", [4, 2, 128, 256], bf16)
    gWlT = din("gWlT", [4, 2, 128, 256], bf16)
    gWrT = din("gWrT", [4, 2, 128, 256], bf16)
    gawT = din("gawT", [4, 2, 128, 256], bf16)
    gowaT = din("gowaT", [4, 2, 128, 256], bf16)
    gowsT = din("gowsT", [4, 2, 128, 256], bf16)
    gbiasT = din("gbiasT", [1, 3072], bf16)
    rbT = din("rbT", [1, 1024], bf16)
    zrow = din("zrow", [1, 256], bf16)
    aggT = din("aggT", [8, 128, 256], bf16)
    aggbT = din("aggbT", [1, 256], bf16)
    decT = din("decT", [2, 128, 4], bf16)
    decb4 = din("decb4", [4, 2], f32)
    hmdd = din("hmdd", [4, 112, 48], bf16)
    Tm = din("Tm", [112, 48], f32)
    ones48 = din("ones48", [112, 128], bf16)
    ones2 = din("ones2", [2, 48], bf16)
    ident = din("ident", [128, 128], bf16)
    eye48 = din("eye48", [48, 48], f32)

    out = nc.dram_tensor("bbox_out", [8, 8, 6, 4], f32, kind="ExternalOutput")
    if dump:
        dbg_mdd = nc.dram_tensor("dbg_mdd", [112, 192], bf16, kind="ExternalOutput")
        dbg_uvt = nc.dram_tensor("dbg_uvt", [112, 1024], bf16, kind="ExternalOutput")
        dbg_cs = nc.dram_tensor("dbg_cs", [128, 384], bf16, kind="ExternalOutput")
        dbg_st = nc.dram_tensor("dbg_st", [128, 480], bf16, kind="ExternalOutput")
        dbg_x = nc.dram_tensor("dbg_x", [128, 384], bf16, kind="ExternalOutput")
        dbg_a = nc.dram_tensor("dbg_a", [128, 384], bf16, kind="ExternalOutput")

    with tile.TileContext(nc) as tc, ExitStack() as ctx:
        # ---- persistent pools ----
        wp = ctx.enter_context(tc.tile_pool(name="w", bufs=1))
        sp = ctx.enter_context(tc.tile_pool(name="state", bufs=1))

        def load(dram_t, shape, dtype, src_ap=None):
            t = wp.tile(shape, dtype, tag=dram_t.name)
            if src_ap is None:
                nc.sync.dma_start(t[:], dram_t[:, :])
            else:
                # src_ap dims [p, d0, d1, ...]; dst = t reshaped to match
                dims = [c for _, c in src_ap.ap[1:]]
                spec = " ".join(f"d{i}" for i in range(len(dims)))
                kw = {f"d{i}": dims[i] for i in range(len(dims) - 1)}
                dv = t[:].rearrange(f"p ({spec}) -> p {spec}", **kw)
                nc.sync.dma_start(dv, src_ap)
            return t

        # conv-critical loads first so im2col g0 isn't queued behind ~4MB of
        # GNN weights; everything else loads mid-body, overlapped with conv.
        w1_s = load(w1, [23, 256] if FP8_CONV else [45, 128], f8)
        b1_s = load(b1, [128, 1], f32)
        if FP8_CONV:
            w2p_s = load(w2p, [128, 768], f8, w2p[:].rearrange("q p m -> p q m"))
        else:
            w2a_s = load(w2a, [128, 3 * 128], f8, w2a[:].rearrange("d p m -> p d m"))
            w2b_s = load(w2b, [128, 3 * 128], f8, w2b[:].rearrange("d p m -> p d m"))
        w2c_s = load(w2c, [64, 3 * 128], f8, w2c[:].rearrange("d p m -> p d m"))
        w2cn_s = load(w2cn, [64, 3 * 128], f8, w2cn[:].rearrange("d p m -> p d m"))
        b2_s = load(b2, [128, 1], f32)
        ident_s = load(ident, [128, 128], bf16)

        class _LW: pass
        lw = _LW()

        def loadg(t):  # [4,2,128,256] -> [128, 4*512]
            return load(t, [128, 2048], bf16, t[:].rearrange("h k p m -> p h k m"))

        # GNN/fc weight loads split into per-conv-group chunks so the DMA
        # overlaps the conv stage instead of stalling fc0/rollout-0, while
        # never queueing ahead of that group's conv-critical im2col/wroi
        def late_c0():
            lw.fc0t_s = load(fc0t, [64, 16 * 256], bf16,
                                  fc0t[:].rearrange("t p m -> p t m"))
            lw.fc0b_s = load(fc0b, [128, 2], f32)
            lw.coor_s = load(coor, [2, 192], bf16)
            lw.fc0ct_s = load(fc0ct, [2, 256], bf16)
            lw.fc0cb_s = load(fc0cb, [128, 2], f32)
            lw.fc1ct_s = load(fc1ct, [128, 512], bf16,
                                   fc1ct[:].rearrange("k p m -> p k m"))
            lw.fc1cb_s = load(fc1cb, [128, 2], f32)
            lw.redoT_s = load(redoT, [128, 512], bf16,
                                   redoT[:].rearrange("k p m -> p k m"))
            lw.redeT_s = load(redeT, [128, 512], bf16,
                                   redeT[:].rearrange("k p m -> p k m"))
            lw.redb_s = load(redb, [128, 2], f32)
            for m in range(4):
                nc.sync.dma_start(mdd_t[m][:], hmdd[m])
            for m in range(4, 11):
                nc.gpsimd.memset(mdd_t[m][32:64, :], 0.0)

        def late_c1():
            lw.gswT_s, lw.gWlT_s, lw.gWrT_s = loadg(gswT), loadg(gWlT), loadg(gWrT)
            lw.gbiasT_s = load(gbiasT, [1, 3072], bf16)
            lw.rbT_s = load(rbT, [1, 1024], bf16)
            lw.zrow_s = load(zrow, [1, 256], bf16)
            lw.Tm_s = load(Tm, [112, 48], f32)
            lw.ones48_s = load(ones48, [112, 128], bf16)
            lw.ones2_s = load(ones2, [2, 48], bf16)
            lw.eye48_s = load(eye48, [48, 48], f32)

        def late_c2():
            lw.gawT_s, lw.gowaT_s = loadg(gawT), loadg(gowaT)

        def late_c3():
            lw.gowsT_s = loadg(gowsT)
            lw.aggT_s = load(aggT, [128, 2048], bf16,
                                  aggT[:].rearrange("k p m -> p k m"))
            lw.aggbT_s = load(aggbT, [1, 256], bf16)
            lw.decT_s = load(decT, [128, 8], bf16,
                                  decT[:].rearrange("k p m -> p k m"))
            lw.decb4_s = load(decb4, [4, 2], f32)

        late_chunks = [late_c0, late_c1, late_c2, late_c3]


        # mask/ddiag slots [112,48]: rows 0-47 diag(deg), 48-63 zero,
        # 64-111 mask (engine writes must start at partition 0/32/64/96)
        mdd_t = [sp.tile([112, 48], bf16, name=f"mdd{m}", tag=f"mdd{m}") for m in range(11)]
        # per-head [u+rb; 0; v] lhsT tiles (rel bias folded in via K=1 matmul)
        uvt = [sp.tile([112, 256], bf16, name=f"uvt{k}", tag=f"uvt{k}") for k in range(4)]

        st = [sp.tile([128, 96], bf16, name=f"st{m}", tag=f"st{m}") for m in range(12)]
        bbox_sb = sp.tile([4, 384], f32, tag="bbox")
        poolT = sp.tile([64, 3072], bf16, tag="poolT")

        def stages():
            if stage == 'setup':
                return

            # ================= conv stage =================
            with ExitStack() as cvx:
                imcp = cvx.enter_context(tc.tile_pool(name="imc", bufs=2))
                f1p = cvx.enter_context(tc.tile_pool(name="f1", bufs=2))
                c1ps = cvx.enter_context(tc.tile_pool(name="c1ps", bufs=3, space="PSUM"))
                c2ps = cvx.enter_context(tc.tile_pool(name="c2ps", bufs=2, space="PSUM"))
                tps = cvx.enter_context(tc.tile_pool(name="tps", bufs=1, space="PSUM"))
                pps = cvx.enter_context(tc.tile_pool(name="pps", bufs=1, space="PSUM"))
                cfps = cvx.enter_context(tc.tile_pool(name="cfps", bufs=1, space="PSUM"))
                f2p = cvx.enter_context(tc.tile_pool(name="f2", bufs=3))
                wrp = cvx.enter_context(tc.tile_pool(name="wr", bufs=2))

                GC = IMG_GRP * 2048
                for g in range(NGRP):
                    if FP8_CONV:
                        imc = imcp.tile([23, 2 * IMG_GRP * 2048], f8, tag="imc")
                        imv = imc[:].rearrange("p (i n) -> p i n", i=2)
                        nc.sync.dma_start(imv[:, 0, :], im2col[:, g * GC:(g + 1) * GC])
                        nc.sync.dma_start(imv[:, 1, :],
                                          im2col[:, 65536 + g * GC:65536 + (g + 1) * GC])
                    else:
                        imc = imcp.tile([45, IMG_GRP * 2048], f8, tag="imc")
                        nc.sync.dma_start(imc[:], im2col[:, g * GC:(g + 1) * GC])
                    f1 = f1p.tile([128, IMG_GRP * IMGF], f8, tag="f1")
                    # layout (py, y33, x32): x tight so the conv2 column walk
                    # collapses to one AP dim; y=32 is a zero row for the dy2
                    # taps (gC's x-wrap at j2=15 is cancelled by corrections)
                    f1h = f1[:].rearrange("p (i py y x) -> p i py y x", i=IMG_GRP, py=2, y=33, x=32)
                    nc.gpsimd.memset(f1h[:, :, :, 32:33, :], 0.0)
                    for i in range(IMG_GRP):
                        # conv1: 4 matmuls of [45,128]x[45,512], each its own
                        # 1-bank psum group; evac relu+bias per group covers
                        # both py phases (psum cols (py2, y8, j32) -> f1)
                        for g4 in range(4):
                            ps = c1ps.tile([128, 512], f32, tag="c1")
                            off = i * 2048 + g4 * 512
                            if FP8_CONV:
                                rhs = bass.AP(imc[:].tensor, off,
                                              [imc[:].ap[0], [16384, 2], [1, 512]])
                                nc.tensor.matmul(ps[:],
                                                 lhsT=w1_s[:].rearrange(
                                                     "p (i m) -> p i m", i=2),
                                                 rhs=rhs, start=True, stop=True,
                                                 perf_mode=mybir.MatmulPerfMode.DoubleRow)
                            else:
                                nc.tensor.matmul(ps[:],
                                                 lhsT=w1_s[:],
                                                 rhs=imc[:, off:off + 512],
                                                 start=True, stop=True)
                            ps_t = ps[:]
                            src = bass.AP(ps_t.tensor, ps_t.offset,
                                          [ps_t.ap[0], [32, 2], [64, 8], [1, 32]])
                            dst = f1h[:, i, :, 8 * g4:8 * g4 + 8, 0:32]
                            if g4 % 2 == 0:
                                nc.vector.tensor_scalar(
                                    out=dst, in0=src, scalar1=b1_s[:, 0:1],
                                    scalar2=0.0, op0=OP.add, op1=OP.max)
                            else:
                                nc.scalar.activation(out=dst, in_=src,
                                                     func=AF.Relu, bias=b1_s[:, 0:1])
                    for i in range(IMG_GRP):
                        # conv2: 9 matmuls -> psum [128, 512] cols (oy2 32, j2 16)
                        wr = wrp.tile([128, 8 * 96], bf16, tag="wr")
                        nc.sync.dma_start(wr[:], wroi[g][:, i * 768:(i + 1) * 768])
                        ps = c2ps.tile([128, 512], f32, tag="c2")
                        fb = f1[:]          # free layout (i, py 2, y 33, x 32)
                        ib = i * IMGF       # image base offset
                        PYS = 33 * 32       # py plane stride

                        def c2walk(py, yo, x0, pair=None):
                            # collapsed column walk: psum col (oy2, j2) reads
                            # f1[py, yo+oy2, x0+2*j2] = base + 2*(oy2*16+j2)
                            off = ib + py * PYS + yo * 32 + x0
                            dims = [fb.ap[0]]
                            if pair is not None:
                                dims.append([pair, 2])
                            dims.append([2, 512])
                            return bass.AP(fb.tensor, fb.offset + off, dims)

                        if FP8_CONV:
                            # fp8 DoubleRow: gA/gB dy0+dy1 paired across py
                            # planes (stride PYS), gA-dy2+gB-dy2 paired
                            # across x (stride 1)
                            DR = mybir.MatmulPerfMode.DoubleRow

                            def w2pair(q):
                                return w2p_s[:, q * 256:(q + 1) * 256].rearrange(
                                    "p (i m) -> p i m", i=2)
                            nc.tensor.matmul(ps[:], lhsT=w2pair(0),
                                             rhs=c2walk(0, 0, 0, pair=PYS),
                                             start=True, stop=False, perf_mode=DR)
                            nc.tensor.matmul(ps[:], lhsT=w2pair(1),
                                             rhs=c2walk(0, 0, 1, pair=PYS),
                                             start=False, stop=False, perf_mode=DR)
                            nc.tensor.matmul(ps[:], lhsT=w2pair(2),
                                             rhs=c2walk(0, 1, 0, pair=1),
                                             start=False, stop=False, perf_mode=DR)
                        else:
                            for dy, (py, yo) in enumerate([(0, 0), (1, 0), (0, 1)]):
                                for x0, wsel in ((0, w2a_s), (1, w2b_s)):
                                    nc.tensor.matmul(ps[:],
                                                     lhsT=wsel[:, dy * 128:(dy + 1) * 128],
                                                     rhs=c2walk(py, yo, x0),
                                                     start=(dy == 0 and x0 == 0),
                                                     stop=False)
                        # gC taps (K=64, x=2j2+2): full walks; the j2=15
                        # column wrongly reads f1[py, y+1, 0] instead of the
                        # zero pad -- negated-weight matmuls accumulate the
                        # cancellation into c2f, added into the j2=15 strip
                        # by the (otherwise idle) gpsimd engine
                        for dy, (py, yo) in enumerate([(0, 0), (1, 0), (0, 1)]):
                            nc.tensor.matmul(ps[:], lhsT=w2c_s[:, dy * 128:(dy + 1) * 128],
                                             rhs=c2walk(py, yo, 2)[0:64],
                                             start=False, stop=(dy == 2))
                        c2f = cfps.tile([128, 32], f32, tag="c2f")
                        for dy, (py, yo) in enumerate([(0, 0), (1, 0), (0, 1)]):
                            off = ib + py * PYS + yo * 32 + 32
                            rhs = bass.AP(fb.tensor, fb.offset + off,
                                          [fb.ap[0], [32, 32]])[0:64]
                            nc.tensor.matmul(c2f[:], lhsT=w2cn_s[:, dy * 128:(dy + 1) * 128],
                                             rhs=rhs, start=(dy == 0),
                                             stop=(dy == 2))
                        c2fs = f2p.tile([128, 32], bf16, tag="c2fs")
                        nc.scalar.activation(out=c2fs[:], in_=c2f[:], func=AF.Copy)
                        pst = ps[:]
                        strip15 = bass.AP(pst.tensor, pst.offset + 15,
                                          [pst.ap[0], [16, 32]])
                        nc.vector.tensor_tensor(out=strip15, in0=strip15,
                                                in1=c2fs[:], op=OP.add)
                        f2s = f2p.tile([128, 512], bf16, tag="f2s")
                        if i % 2 == 0:
                            nc.vector.tensor_scalar(out=f2s[:], in0=ps[:], scalar1=b2_s[:, 0:1],
                                                    scalar2=0.0, op0=OP.add, op1=OP.max)
                        else:
                            nc.scalar.activation(out=f2s[:], in_=ps[:], func=AF.Relu,
                                                 bias=b2_s[:, 0:1])
                        tp = tps.tile([128, 512], bf16, tag="tp")
                        for b in range(4):
                            nc.tensor.transpose(tp[:, b * 128:(b + 1) * 128],
                                                f2s[:, b * 128:(b + 1) * 128], ident_s[:])
                        f2t = f2p.tile([128, 512], bf16, tag="f2t")
                        if i % 2 == 0:
                            nc.scalar.activation(out=f2t[:], in_=tp[:], func=AF.Copy)
                        else:
                            nc.vector.tensor_copy(out=f2t[:], in_=tp[:])
                        # RoIAlign as matmul: pool_ps[c, n*16+pt] = sum over
                        # pixel chunks (b, px) of f2t-slice^T @ wroi-slice
                        img = g * IMG_GRP + i
                        pool_ps = pps.tile([64, 96], f32, tag="pool")
                        for c in range(8):
                            b, px = divmod(c, 2)
                            nc.tensor.matmul(
                                pool_ps[:],
                                lhsT=f2t[:, b * 128 + px * 64:b * 128 + px * 64 + 64],
                                rhs=wr[:, c * 96:c * 96 + 96],
                                start=(c == 0), stop=(c == 7))
                        if i % 2 == 0:
                            nc.scalar.activation(out=poolT[:, img * 96:(img + 1) * 96],
                                                 in_=pool_ps[:], func=AF.Copy)
                        else:
                            nc.vector.tensor_copy(out=poolT[:, img * 96:(img + 1) * 96],
                                                  in_=pool_ps[:])
                    if not getattr(lw, 'done', False):
                        late_chunks[g]()
                        if g == NGRP - 1:
                            lw.done = True
            if stage == 'conv':
                return

            # ================= fc0 + emb + red =================
            with ExitStack() as gx:
                ops = gx.enter_context(tc.tile_pool(name="ops", bufs=2, space="PSUM"))

                obj = sp.tile([128, 384], bf16, tag="obj")
                pview = poolT[:].rearrange("p (r t) -> p t r", t=16)
                for m2 in range(2):
                    ps = ops.tile([128, 192], f32, tag="obj")
                    for pt_i in range(16):
                        nc.tensor.matmul(ps[:], lhsT=lw.fc0t_s[:, pt_i * 256 + m2 * 128:
                                                            pt_i * 256 + m2 * 128 + 128],
                                         rhs=pview[:, pt_i, :],
                                         start=(pt_i == 0), stop=(pt_i == 15))
                    nc.scalar.activation(out=obj[:, m2 * 192:(m2 + 1) * 192], in_=ps[:],
                                         func=AF.Relu, bias=lw.fc0b_s[:, m2:m2 + 1])
                emb1 = sp.tile([128, 384], bf16, tag="emb1")
                for m2 in range(2):
                    ps = ops.tile([128, 192], f32, tag="emb")
                    nc.tensor.matmul(ps[:], lhsT=lw.fc0ct_s[:, m2 * 128:(m2 + 1) * 128],
                                     rhs=lw.coor_s[:], start=True, stop=True)
                    nc.scalar.activation(out=emb1[:, m2 * 192:(m2 + 1) * 192], in_=ps[:],
                                         func=AF.Relu, bias=lw.fc0cb_s[:, m2:m2 + 1])
                emb2 = sp.tile([128, 384], bf16, tag="emb2")
                for m2 in range(2):
                    ps = ops.tile([128, 192], f32, tag="emb")
                    for kc in range(2):
                        nc.tensor.matmul(ps[:], lhsT=lw.fc1ct_s[:, kc * 256 + m2 * 128:
                                                             kc * 256 + m2 * 128 + 128],
                                         rhs=emb1[:, kc * 192:(kc + 1) * 192],
                                         start=(kc == 0), stop=(kc == 1))
                    nc.scalar.activation(out=emb2[:, m2 * 192:(m2 + 1) * 192], in_=ps[:],
                                         func=AF.Relu, bias=lw.fc1cb_s[:, m2:m2 + 1])
                o2 = sp.tile([128, 384], bf16, tag="o2")
                for m2 in range(2):
                    ps = ops.tile([128, 192], f32, tag="o2")
                    for kc in range(2):
                        nc.tensor.matmul(ps[:], lhsT=lw.redoT_s[:, kc * 256 + m2 * 128:
                                                             kc * 256 + m2 * 128 + 128],
                                         rhs=obj[:, kc * 192:(kc + 1) * 192],
                                         start=(kc == 0), stop=False)
                    for kc in range(2):
                        nc.tensor.matmul(ps[:], lhsT=lw.redeT_s[:, kc * 256 + m2 * 128:
                                                             kc * 256 + m2 * 128 + 128],
                                         rhs=emb2[:, kc * 192:(kc + 1) * 192],
                                         start=False, stop=(kc == 1))
                    nc.scalar.activation(out=o2[:, m2 * 192:(m2 + 1) * 192], in_=ps[:],
                                         func=AF.Relu, bias=lw.redb_s[:, m2:m2 + 1])
                # initial states: s_m [128, 96] cols m2*48 + b*6 + n  <- o2 cols m2*192 + b*24 + m*6 + n
                o2v = o2[:].rearrange("p (m2 b t n) -> p m2 b t n", m2=2, b=8, t=4)
                for m in range(4):
                    nc.vector.tensor_copy(
                        out=st[m][:].rearrange("p (m2 b n) -> p m2 b n", m2=2, b=8),
                        in_=o2v[:, :, :, m, :])

            if stage.startswith('gather'):
                return

            # ================= GNN rollouts =================
            with ExitStack() as rx:
                gps = rx.enter_context(tc.tile_pool(name="gps", bufs=4, space="PSUM"))
                vps = rx.enter_context(tc.tile_pool(name="vps", bufs=2, space="PSUM"))
                sps = rx.enter_context(tc.tile_pool(name="sps", bufs=2, space="PSUM"))
                hb = rx.enter_context(tc.tile_pool(name="hbuf", bufs=3))

                def emit_uv(rr, ks):
                    # uv_ps rows 0-47 = u+rb = s@Wl^T + rb (rb via K=1 matmul),
                    # rows 64-111 = v = s@Wr^T; contiguous accumulation group
                    # per partition region; copies alternate DVE/Act
                    for k in ks:
                        s = st[rr + k]
                        uv_ps = vps.tile([112, 256], f32, tag="v")
                        # zero rows 32-63 first (write base must be 0/32/64; the
                        # u matmuls below re-cover 32-47 with real data).
                        # skip_group_check: self-contained start+stop write
                        # whose partition-offset aliases CoreSim's zero-region
                        # tracker against the row-0/row-64 groups
                        nc.tensor.matmul(uv_ps[32:64, :], lhsT=lw.ones2_s[0:1, 0:32],
                                         rhs=lw.zrow_s[:], start=True, stop=True,
                                         skip_group_check=True)
                        for kc in range(2):
                            nc.tensor.matmul(uv_ps[0:48, :], lhsT=s[:, kc * 48:kc * 48 + 48],
                                             rhs=lw.gWlT_s[:, k * 512 + kc * 256:
                                                        k * 512 + (kc + 1) * 256],
                                             start=(kc == 0), stop=False)
                        nc.tensor.matmul(uv_ps[0:48, :], lhsT=lw.ones2_s[0:1, :],
                                         rhs=lw.rbT_s[:, k * 256:(k + 1) * 256],
                                         start=False, stop=True)
                        for kc in range(2):
                            nc.tensor.matmul(uv_ps[64:112, :], lhsT=s[:, kc * 48:kc * 48 + 48],
                                             rhs=lw.gWrT_s[:, k * 512 + kc * 256:
                                                        k * 512 + (kc + 1) * 256],
                                             start=(kc == 0), stop=(kc == 1),
                                             skip_group_check=True)
                        if k % 2 == 0:
                            nc.vector.tensor_copy(out=uvt[k][:], in_=uv_ps[:])
                        else:
                            nc.scalar.activation(out=uvt[k][:], in_=uv_ps[:], func=AF.Copy)

                def emit_x(rr, k):
                    m = rr + k
                    s = st[rr + k]
                    # x = rel + deg*(u+rb) + self-dynamics; one contiguous psum
                    # accumulation group per half (interleaved groups in one
                    # bank mis-accumulate): rel first, then sd matmuls
                    x_ps = gps.tile([128, 96], f32, tag="g")
                    for m2 in range(2):
                        nc.tensor.matmul(x_ps[:, m2 * 48:m2 * 48 + 48],
                                         lhsT=uvt[k][:, m2 * 128:(m2 + 1) * 128],
                                         rhs=mdd_t[m][:], start=True, stop=False)
                        for kc in range(2):
                            lo = k * 512 + kc * 256 + m2 * 128
                            nc.tensor.matmul(x_ps[:, m2 * 48:m2 * 48 + 48],
                                             lhsT=lw.gswT_s[:, lo:lo + 128],
                                             rhs=s[:, kc * 48:kc * 48 + 48],
                                             start=False, stop=False)
                        nc.tensor.matmul(x_ps[:, m2 * 48:m2 * 48 + 48],
                                         lhsT=lw.gbiasT_s[:, k * 256 + m2 * 128:
                                                          k * 256 + m2 * 128 + 128],
                                         rhs=lw.ones2_s[0:1, :],
                                         start=False, stop=True)
                    return x_ps

                def evac_plain(ps_t, k, tag, relu):
                    # bias already accumulated on PE; single whole-tile copy,
                    # alternating engines by head parity
                    sb = hb.tile([128, 96], bf16, tag=f"{tag}{k}")
                    if relu:
                        if k % 2 == 0:
                            nc.scalar.activation(out=sb[:], in_=ps_t[:],
                                                 func=AF.Relu, bias=0.0)
                        else:
                            nc.vector.tensor_scalar(out=sb[:], in0=ps_t[:],
                                                    scalar1=0.0, scalar2=None,
                                                    op0=OP.max)
                    else:
                        if k % 2 == 0:
                            nc.vector.tensor_copy(out=sb[:], in_=ps_t[:])
                        else:
                            nc.scalar.activation(out=sb[:], in_=ps_t[:], func=AF.Copy)
                    return sb

                def emit_a(k, x_sbs):
                    a_ps = gps.tile([128, 96], f32, tag="g")
                    for m2 in range(2):
                        for kc in range(2):
                            lo = k * 512 + kc * 256 + m2 * 128
                            nc.tensor.matmul(a_ps[:, m2 * 48:m2 * 48 + 48],
                                             lhsT=lw.gawT_s[:, lo:lo + 128],
                                             rhs=x_sbs[k][:, kc * 48:kc * 48 + 48],
                                             start=(kc == 0), stop=False)
                        nc.tensor.matmul(a_ps[:, m2 * 48:m2 * 48 + 48],
                                         lhsT=lw.gbiasT_s[:, 1024 + k * 256 + m2 * 128:
                                                          1024 + k * 256 + m2 * 128 + 128],
                                         rhs=lw.ones2_s[0:1, :],
                                         start=False, stop=True)
                    return a_ps

                def emit_o(rr, k, a_sbs):
                    s = st[rr + k]
                    o_ps = gps.tile([128, 96], f32, tag="g")
                    for m2 in range(2):
                        for kc in range(2):
                            lo = k * 512 + kc * 256 + m2 * 128
                            nc.tensor.matmul(o_ps[:, m2 * 48:m2 * 48 + 48],
                                             lhsT=lw.gowaT_s[:, lo:lo + 128],
                                             rhs=a_sbs[k][:, kc * 48:kc * 48 + 48],
                                             start=(kc == 0), stop=False)
                            nc.tensor.matmul(o_ps[:, m2 * 48:m2 * 48 + 48],
                                             lhsT=lw.gowsT_s[:, lo:lo + 128],
                                             rhs=s[:, kc * 48:kc * 48 + 48],
                                             start=False, stop=False)
                        nc.tensor.matmul(o_ps[:, m2 * 48:m2 * 48 + 48],
                                         lhsT=lw.gbiasT_s[:, 2048 + k * 256 + m2 * 128:
                                                          2048 + k * 256 + m2 * 128 + 128],
                                         rhs=lw.ones2_s[0:1, :],
                                         start=False, stop=True)
                    return o_ps

                emit_uv(0, range(4))
                tail = [None]
                for rr in range(8):
                    # stage-major emission; head 3 trails heads 0-2 by one
                    # stage so the previous rollout's dec+mask chain (emitted
                    # between, as `tail`) can resolve off the PE critical path
                    x_pss, x_sbs, a_pss, a_sbs, o_pss, cs = [], [], [], [], [], []
                    for k in range(3):
                        x_pss.append(emit_x(rr, k))
                    for k in range(3):
                        x_sbs.append(evac_plain(x_pss[k], k, "x", False))
                    if tail[0] is not None:
                        tail[0]()
                        tail[0] = None
                    for k in range(3):
                        a_pss.append(emit_a(k, x_sbs))
                    x_pss.append(emit_x(rr, 3))
                    x_sbs.append(evac_plain(x_pss[3], 3, "x", False))
                    for k in range(3):
                        a_sbs.append(evac_plain(a_pss[k], k, "a", True))
                    for k in range(3):
                        o_pss.append(emit_o(rr, k, a_sbs))
                    a_pss.append(emit_a(3, x_sbs))
                    a_sbs.append(evac_plain(a_pss[3], 3, "a", True))
                    for k in range(3):
                        cs.append(evac_plain(o_pss[k], k, "cs", True))
                    o_pss.append(emit_o(rr, 3, a_sbs))
                    cs.append(evac_plain(o_pss[3], 3, "cs", True))
                    if dump and rr == 0:
                        for k in range(4):
                            nc.sync.dma_start(dbg_uvt[:, k * 256:(k + 1) * 256], uvt[k][:])
                            nc.sync.dma_start(dbg_x[:, k * 96:(k + 1) * 96], x_sbs[k][:])
                            nc.sync.dma_start(dbg_a[:, k * 96:(k + 1) * 96], a_sbs[k][:])
                    if rr < 7:
                        emit_uv(rr + 1, range(3))
                    g_ps = gps.tile([128, 96], f32, tag="g")
                    for m2 in range(2):
                        n = 0
                        for k in range(4):
                            for kc in range(2):
                                lo = (k * 2 + kc) * 256 + m2 * 128
                                nc.tensor.matmul(g_ps[:, m2 * 48:m2 * 48 + 48],
                                                 lhsT=lw.aggT_s[:, lo:lo + 128],
                                                 rhs=cs[k][:, kc * 48:kc * 48 + 48],
                                                 start=(n == 0), stop=False)
                                n += 1
                        nc.tensor.matmul(g_ps[:, m2 * 48:m2 * 48 + 48],
                                         lhsT=lw.aggbT_s[:, m2 * 128:(m2 + 1) * 128],
                                         rhs=lw.ones2_s[0:1, :],
                                         start=False, stop=True)
                    if dump and rr == 0:
                        for k in range(4):
                            nc.sync.dma_start(dbg_cs[:, k * 96:(k + 1) * 96], cs[k][:])
                            nc.sync.dma_start(dbg_mdd[:, k * 48:(k + 1) * 48], mdd_t[k][:])
                    s_new = st[rr + 4]
                    nc.vector.tensor_copy(out=s_new[:], in_=g_ps[:])
                    if rr < 7:
                        emit_uv(rr + 1, [3])

                    def make_tail(rr, s_new):
                        def tail_fn():
                            d_ps = sps.tile([4, 48], f32, tag="s")
                            for kc in range(2):
                                nc.tensor.matmul(d_ps[:], lhsT=lw.decT_s[:, kc * 4:kc * 4 + 4],
                                                 rhs=s_new[:, kc * 48:kc * 48 + 48],
                                                 start=(kc == 0), stop=(kc == 1))
                            bbv = bbox_sb[:].rearrange("f (b q) -> f b q", b=8)[:, :, rr * 6:rr * 6 + 6]
                            nc.vector.tensor_scalar(out=bbv, in0=d_ps[:],
                                                    scalar1=lw.decb4_s[:, 0:1],
                                                    scalar2=None, op0=OP.add)
                            if rr >= 7:
                                return
                            m = rr + 4
                            d2_ps = sps.tile([2, 48], f32, tag="s")
                            for kc in range(2):
                                nc.tensor.matmul(d2_ps[:], lhsT=lw.decT_s[:, kc * 4 + 2:kc * 4 + 4],
                                                 rhs=s_new[:, kc * 48:kc * 48 + 48],
                                                 start=(kc == 0), stop=(kc == 1))
                            # coorb / -2*coorb / coorb^2 all read d2_ps directly
                            # (coorb is only needed as a matmul lhsT)
                            coorb = hb.tile([2, 48], bf16, tag="coorb")
                            nc.vector.tensor_scalar(out=coorb[:], in0=d2_ps[:],
                                                    scalar1=lw.decb4_s[0:2, 1:2],
                                                    scalar2=None, op0=OP.add)
                            cm2 = hb.tile([2, 48], bf16, tag="cm2")
                            nc.vector.tensor_scalar(out=cm2[:], in0=d2_ps[:],
                                                    scalar1=lw.decb4_s[0:2, 1:2],
                                                    scalar2=-2.0, op0=OP.add,
                                                    op1=OP.mult)
                            sq = hb.tile([2, 48], bf16, tag="sq")
                            nc.scalar.activation(out=sq[:], in_=d2_ps[:], func=AF.Square,
                                                 bias=lw.decb4_s[0:2, 1:2])
                            m_ps = sps.tile([112, 48], f32, tag="s")
                            nc.tensor.matmul(m_ps[64:112, :], lhsT=coorb[:], rhs=cm2[:],
                                             start=True, stop=False,
                                             skip_group_check=True)
                            nc.tensor.matmul(m_ps[64:112, :], lhsT=sq[:], rhs=lw.ones2_s[:],
                                             start=False, stop=False,
                                             skip_group_check=True)
                            nc.tensor.matmul(m_ps[64:112, :], lhsT=lw.ones2_s[:], rhs=sq[:],
                                             start=False, stop=True,
                                             skip_group_check=True)
                            nc.vector.tensor_tensor(out=mdd_t[m][64:112, :], in0=m_ps[64:112, :],
                                                    in1=lw.Tm_s[64:112, :], op=OP.is_le)
                            dd_ps = sps.tile([128, 48], f32, tag="s")
                            nc.tensor.matmul(dd_ps[:], lhsT=lw.ones48_s[64:112, :],
                                             rhs=mdd_t[m][64:112, :], start=True, stop=True)
                            nc.vector.tensor_tensor(out=mdd_t[m][0:48, :], in0=lw.eye48_s[:],
                                                    in1=dd_ps[0:48, :], op=OP.mult)
                        return tail_fn
                    tail[0] = make_tail(rr, s_new)
                tail[0]()
                tail[0] = None
                if dump:
                    for m in range(5):
                        nc.sync.dma_start(dbg_st[:, m * 96:(m + 1) * 96], st[m][:])
        for _rep in range(nrep):
            stages()
        if stage != 'full':
            nc.gpsimd.memset(bbox_sb[:], 0.0)
        nc.sync.dma_start(
            out[:].rearrange("b rr n f -> f (b rr n)"), bbox_sb[:])
    return nc


_NC = None

def _get_nc():
    global _NC
    if _NC is None:
        nc = bass.Bass()
        build(nc)
        split_drain_waits(nc)
        _NC = nc
    return _NC


def kernel(**inputs):
    nc = _get_nc()
    inputs = {k: np.asarray(v) for k, v in inputs.items()}
    maps = [make_core_inputs(inputs, s) for s in range(NCORE)]
    res = run_bass_kernel_spmd(nc, maps, core_ids=list(range(NCORE)))
    out = np.concatenate([res.results[s]["bbox_out"] for s in range(NCORE)], 0)
    return out.astype(np.float32)

